# revision 1
# baseline (speedup 1.0000x reference)
"""Bass/Tile kernel body for SSD postprocess (one image per core).

HW constraint honored throughout: indirect DMA supports exactly ONE offset per
partition per instruction (descriptor = contiguous trailing dims of in_).

build_kernel(tc, outs, ins, dbg, stop_after) emits the full per-core program.
  ins:  lg [8832, 91] f32 (pad rows zero), rl [8832,4] f32, an [8832,4] f32
  outs: out [200, 6] f32
"""
import numpy as np
import concourse.bass as bass
import concourse.mybir as mybir
from concourse.masks import make_identity

P = 128
NT = 69            # anchor tiles (8832 = 69*128)
NC = 91
C1 = 90
NANC = NT * P      # 8832
NCH = NANC // 8    # 1104 consecutive-anchor chunks per class
MCH = 24           # top chunks per class
MEL = 24           # top elements per class
MUSE = 22          # columns merged globally (90*22 = 1980)
CAP = 2048
OUTN = 200
ND = C1 * MCH      # 2160 descriptors for d-major gathers
NDP = 17 * P       # 2176 padded
NEG = -1e30
LN01 = float(np.log(0.01))
BBOX_CLIP = float(np.log(1000.0 / 16.0))

F32 = mybir.dt.float32
U32 = mybir.dt.uint32
OP = mybir.AluOpType
AF = mybir.ActivationFunctionType
AX = mybir.AxisListType


def build_kernel(tc, outs, ins, dbg=None, stop_after="full", reps=1):
    nc = tc.nc
    dbg = dbg or {}
    STAGES = ["s1", "s2", "s2b", "s3", "s4", "s5", "s6", "s7", "s8", "s9", "full"]
    LIMIT = STAGES.index(stop_after)

    def past(stage):
        return STAGES.index(stage) > LIMIT

    def dump(name, ap):
        if name in dbg:
            nc.sync.dma_start(dbg[name][:], ap)

    # DRAM scratch (offset-0 tensors; indirect DMA requires offset==0)
    TDC = nc.dram_tensor("tdc_scratch", [C1 * NCH, 8], F32, kind="Internal").ap()
    BD = nc.dram_tensor("bd_scratch", [NANC, 4], F32, kind="Internal").ap()
    OD1 = nc.dram_tensor("od1_scratch", [91 * MCH], U32, kind="Internal").ap()
    GD1 = nc.dram_tensor("gd1_scratch", [NDP, 8], F32, kind="Internal").ap()
    OD2 = nc.dram_tensor("od2_scratch", [91 * MCH], U32, kind="Internal").ap()
    GD2 = nc.dram_tensor("gd2_scratch", [NDP, 4], F32, kind="Internal").ap()
    AUX = nc.dram_tensor("aux_scratch", [C1 * MUSE, 2], U32, kind="Internal").ap()
    SK = nc.dram_tensor("sk_scratch", [94 * MUSE], F32, kind="Internal").ap()
    SP = nc.dram_tensor("sp_scratch", [94 * MUSE], F32, kind="Internal").ap()
    SOK = nc.dram_tensor("sok_scratch", [256], F32, kind="Internal").ap()
    SOP = nc.dram_tensor("sop_scratch", [256], F32, kind="Internal").ap()
    SOK2 = nc.dram_tensor("sok2_scratch", [256], F32, kind="Internal").ap()
    SOP2 = nc.dram_tensor("sop2_scratch", [256], F32, kind="Internal").ap()

    with tc.tile_pool(name="big", bufs=1) as bp, \
         tc.tile_pool(name="sm", bufs=2) as sm, \
         tc.tile_pool(name="sx", bufs=1) as sx, \
         tc.tile_pool(name="ps", bufs=2, space="PSUM") as ps, \
         tc.tile_pool(name="psb", bufs=4, space="PSUM") as psb:

        def emit():
            # ---------- S1: load logits, log-softmax ----------
            LG = bp.tile([P, NT, NC], F32, tag="LG")
            nc.sync.dma_start(LG[:], ins["lg"][:].rearrange("(t p) c -> p t c", p=P))
            mx = sm.tile([P, NT], F32, tag="mx")
            nc.vector.tensor_reduce(mx[:], LG[:], axis=AX.X, op=OP.max)
            T69 = bp.tile([P, NT, NC], F32, tag="T69")
            nc.vector.tensor_tensor(T69[:], LG[:], mx[:].to_broadcast([P, NT, NC]),
                                    op=OP.subtract)
            E = bp.tile([P, NT, NC], F32, tag="E")
            nc.scalar.activation(E[:], T69[:], AF.Exp)
            Z = sm.tile([P, NT], F32, tag="Z")
            nc.vector.tensor_reduce(Z[:], E[:], axis=AX.X, op=OP.add)
            lnZ = sm.tile([P, NT], F32, tag="lnZ")
            nc.scalar.activation(lnZ[:], Z[:], AF.Ln)
            logZ = sm.tile([P, NT], F32, tag="logZ")
            nc.vector.tensor_tensor(logZ[:], mx[:], lnZ[:], op=OP.add)
            nc.vector.tensor_tensor(T69[:], LG[:], logZ[:].to_broadcast([P, NT, NC]),
                                    op=OP.subtract)
            dump("T", T69[:])

            if past("s1"):
                return
            # ---------- S2: decode boxes ----------
            RL = bp.tile([P, NT, 4], F32, tag="RL")
            nc.sync.dma_start(RL[:], ins["rl"][:].rearrange("(t p) c -> p t c", p=P))
            AN = bp.tile([P, NT, 4], F32, tag="AN")
            nc.sync.dma_start(AN[:], ins["an"][:].rearrange("(t p) c -> p t c", p=P))

            def col(t, k):
                return t[:, :, k]

            aw = sm.tile([P, NT], F32, tag="aw")
            nc.vector.tensor_tensor(aw[:], col(AN, 2), col(AN, 0), op=OP.subtract)
            ah = sm.tile([P, NT], F32, tag="ah")
            nc.vector.tensor_tensor(ah[:], col(AN, 3), col(AN, 1), op=OP.subtract)
            ax = sm.tile([P, NT], F32, tag="ax")
            nc.vector.tensor_scalar(ax[:], aw[:], 0.5, None, op0=OP.mult)
            nc.vector.tensor_tensor(ax[:], ax[:], col(AN, 0), op=OP.add)
            ay = sm.tile([P, NT], F32, tag="ay")
            nc.vector.tensor_scalar(ay[:], ah[:], 0.5, None, op0=OP.mult)
            nc.vector.tensor_tensor(ay[:], ay[:], col(AN, 1), op=OP.add)

            BX = bp.tile([P, NT, 4], F32, tag="BX")
            dx = sm.tile([P, NT], F32, tag="dx")
            nc.vector.tensor_scalar(dx[:], col(RL, 0), 0.1, None, op0=OP.mult)
            px = sm.tile([P, NT], F32, tag="px")
            nc.vector.tensor_tensor(px[:], dx[:], aw[:], op=OP.mult)
            nc.vector.tensor_tensor(px[:], px[:], ax[:], op=OP.add)
            dw = sm.tile([P, NT], F32, tag="dw")
            nc.vector.tensor_scalar(dw[:], col(RL, 2), 0.2, BBOX_CLIP, op0=OP.mult, op1=OP.min)
            ew = sm.tile([P, NT], F32, tag="ew")
            nc.scalar.activation(ew[:], dw[:], AF.Exp)
            pw = sm.tile([P, NT], F32, tag="pw")
            nc.vector.tensor_tensor(pw[:], ew[:], aw[:], op=OP.mult)
            nc.vector.tensor_scalar(pw[:], pw[:], 0.5, None, op0=OP.mult)

            dy = sm.tile([P, NT], F32, tag="dy")
            nc.vector.tensor_scalar(dy[:], col(RL, 1), 0.1, None, op0=OP.mult)
            py = sm.tile([P, NT], F32, tag="py")
            nc.vector.tensor_tensor(py[:], dy[:], ah[:], op=OP.mult)
            nc.vector.tensor_tensor(py[:], py[:], ay[:], op=OP.add)
            dh = sm.tile([P, NT], F32, tag="dh")
            nc.vector.tensor_scalar(dh[:], col(RL, 3), 0.2, BBOX_CLIP, op0=OP.mult, op1=OP.min)
            eh = sm.tile([P, NT], F32, tag="eh")
            nc.scalar.activation(eh[:], dh[:], AF.Exp)
            ph = sm.tile([P, NT], F32, tag="ph")
            nc.vector.tensor_tensor(ph[:], eh[:], ah[:], op=OP.mult)
            nc.vector.tensor_scalar(ph[:], ph[:], 0.5, None, op0=OP.mult)

            tq = sm.tile([P, NT], F32, tag="tq")
            for k, (ctr, half) in enumerate([(px, pw), (py, ph)]):
                nc.vector.tensor_tensor(tq[:], ctr[:], half[:], op=OP.subtract)
                nc.vector.tensor_scalar(col(BX, k), tq[:], 0.0, 300.0, op0=OP.max, op1=OP.min)
                nc.vector.tensor_tensor(tq[:], ctr[:], half[:], op=OP.add)
                nc.vector.tensor_scalar(col(BX, k + 2), tq[:], 0.0, 300.0, op0=OP.max, op1=OP.min)
            dump("BX", BX[:])

            if past("s2"):
                return
            # ---------- S2b: transpose T to class-major; stage to DRAM ----------
            ident = bp.tile([P, P], F32, tag="ident")
            make_identity(nc, ident[:])
            TCM = bp.tile([C1, NANC], F32, tag="TCM")   # classes 1..90 on partitions 0..89
            for t in range(NT):
                pt = ps.tile([C1, P], F32, tag="trp", space="PSUM")
                nc.tensor.transpose(out=pt[:], in_=T69[:, t, 1:NC], identity=ident[:])
                nc.vector.tensor_copy(TCM[:, t * P:(t + 1) * P], pt[:])
            nc.sync.dma_start(TDC.rearrange("(c h) i -> c (h i)", c=C1), TCM[:])
            nc.sync.dma_start(BD.rearrange("(t p) c -> p t c", p=P), BX[:])
            dump("TCM", TCM[:])

            if past("s2b"):
                return
            # ---------- S3: chunk max (chunk = 8 consecutive anchors) ----------
            CC = bp.tile([C1, NCH], F32, tag="CC")
            nc.vector.tensor_reduce(CC[:], TCM[:].rearrange("c (h i) -> c h i", i=8),
                                    axis=AX.X, op=OP.max)
            dump("CC", CC[:])

            if past("s3"):
                return
            # ---------- S4: top-24 chunks per class ----------
            CHI = bp.tile([C1, MCH], U32, tag="CHI")
            for r in range(3):
                m8 = sm.tile([C1, 8], F32, tag="m8")
                nc.vector.max(out=m8[:], in_=CC[:])
                i8 = sm.tile([C1, 8], U32, tag="i8")
                nc.vector.max_index(out=i8[:], in_max=m8[:], in_values=CC[:])
                nc.vector.tensor_copy(CHI[:, 8 * r:8 * r + 8], i8[:])
                if r < 2:
                    nc.vector.match_replace(out=CC[:], in_to_replace=m8[:], in_values=CC[:],
                                            imm_value=NEG)
            dump("CHI", CHI[:])

            if past("s4"):
                return
            # ---------- S5: gather chunk members via d-major row gathers ----------
            # descriptor d = c*24 + k reads TDC row (c*1104 + CHI[c,k]) = 8 values
            def dmajor_gather(name, offs_cls, od, gd, row_elems, src_rows_ap, out_tile):
                """offs_cls [C1, MCH] u32 row indices; gathers row_elems f32 per row;
                returns via out_tile [C1, MCH*row_elems] (class-major)."""
                ofx = sx.tile([91, MCH], U32, tag=f"ofx_{name}")
                nc.vector.memset(ofx[:], 0)
                nc.vector.tensor_copy(ofx[0:C1, :], offs_cls)
                nc.sync.dma_start(od.rearrange("(c k) -> c k", c=91), ofx[:])
                odb = sx.tile([P, 17], U32, tag=f"odb_{name}")
                nc.sync.dma_start(odb[:], od[0:NDP].rearrange("(t p) -> p t", p=P))
                gvd = sx.tile([P, 17, row_elems], F32, tag=f"gvd_{name}")
                for t in range(17):
                    nc.gpsimd.indirect_dma_start(
                        out=gvd[:, t, :], out_offset=None, in_=src_rows_ap,
                        in_offset=bass.IndirectOffsetOnAxis(ap=odb[:, t:t + 1], axis=0))
                nc.sync.dma_start(gd.rearrange("(t p) i -> p t i", p=P), gvd[:])
                nc.sync.dma_start(
                    out_tile,
                    gd[0:ND, :].rearrange("(c k) i -> c (k i)", c=C1))

            OFF1 = sm.tile([C1, MCH], U32, tag="OFF1")
            CBASE = sm.tile([C1, 1], U32, tag="CBASE")
            nc.gpsimd.iota(CBASE[:], pattern=[[0, 1]], base=0, channel_multiplier=NCH)
            nc.vector.tensor_tensor(OFF1[:], CHI[:], CBASE[:].to_broadcast([C1, MCH]), op=OP.add)
            GV = bp.tile([C1, MCH * 8], F32, tag="GV")
            dmajor_gather("gv", OFF1[:], OD1, GD1, 8, TDC[:], GV[:])
            dump("GV", GV[:])

            # anchor payload = CHI*8 + i (consecutive chunks)
            PAYA = bp.tile([C1, MCH, 8], U32, tag="PAYA")
            A8 = sm.tile([C1, MCH], U32, tag="A8")
            nc.vector.tensor_scalar(A8[:], CHI[:], 3, None, op0=OP.logical_shift_left)
            IOT8 = bp.tile([C1, MCH, 8], U32, tag="IOT8")
            nc.gpsimd.iota(IOT8[:], pattern=[[0, MCH], [1, 8]], base=0, channel_multiplier=0)
            nc.vector.tensor_tensor(PAYA[:], A8[:].to_broadcast([C1, MCH, 8]), IOT8[:], op=OP.add)
            PAYF = bp.tile([C1, MCH * 8], F32, tag="PAYF")
            nc.vector.tensor_copy(PAYF[:], PAYA[:].rearrange("c k i -> c (k i)"))

            if past("s5"):
                return
            # ---------- S6: top-24 elements per class; payloads via value match ----------
            GVK = bp.tile([C1, MCH * 8], F32, tag="GVK")
            nc.vector.tensor_copy(GVK[:], GV[:])
            VAL = bp.tile([C1, MEL], F32, tag="VAL")
            for r in range(3):
                m8b = sm.tile([C1, 8], F32, tag="m8b")
                nc.vector.max(out=m8b[:], in_=GVK[:])
                nc.vector.tensor_copy(VAL[:, 8 * r:8 * r + 8], m8b[:])
                if r < 2:
                    nc.vector.match_replace(out=GVK[:], in_to_replace=m8b[:], in_values=GVK[:],
                                            imm_value=NEG)
            dump("VAL", VAL[:])
            # ANC[c, r] = sum_j PAYA[c,j] * (GV[c,j] == VAL[c,r])  (no ties among used values)
            EQ = sx.tile([C1, MEL, MCH * 8], F32, tag="EQ")
            nc.vector.tensor_tensor(EQ[:], VAL[:].to_broadcast([C1, MEL, MCH * 8]),
                                    GV[:].to_broadcast([C1, MCH * 8, MEL]).rearrange("c a b -> c b a"),
                                    op=OP.is_equal)
            nc.vector.tensor_tensor(EQ[:], EQ[:],
                                    PAYF[:].to_broadcast([C1, MCH * 8, MEL]).rearrange("c a b -> c b a"),
                                    op=OP.mult)
            ANCF = bp.tile([C1, MEL], F32, tag="ANCF")
            nc.vector.tensor_reduce(ANCF[:], EQ[:], axis=AX.X, op=OP.add)
            ANC = bp.tile([C1, MEL], U32, tag="ANC")
            nc.vector.tensor_copy(ANC[:], ANCF[:])
            dump("ANC", ANC[:])

            if past("s6"):
                return
            # ---------- S7: gather boxes (d-major), build A, greedy NMS ----------
            BOXGF = bp.tile([C1, MEL * 4], F32, tag="BOXGF")
            dmajor_gather("bx", ANC[:], OD2, GD2, 4, BD[:], BOXGF[:])
            BOXG = BOXGF[:].rearrange("c (k i) -> c k i", i=4)
            x1 = BOXG[:, :, 0]; y1 = BOXG[:, :, 1]; x2 = BOXG[:, :, 2]; y2 = BOXG[:, :, 3]
            AREA = sm.tile([C1, MEL], F32, tag="AREA")
            wq = sm.tile([C1, MEL], F32, tag="wq")
            nc.vector.tensor_tensor(wq[:], x2, x1, op=OP.subtract)
            nc.vector.tensor_tensor(AREA[:], y2, y1, op=OP.subtract)
            nc.vector.tensor_tensor(AREA[:], AREA[:], wq[:], op=OP.mult)

            def bi(apv):
                return apv.to_broadcast([C1, MEL, MEL])

            def bj(apv):
                return apv.to_broadcast([C1, MEL, MEL]).rearrange("c a b -> c b a")

            AM = bp.tile([C1, MEL, MEL], F32, tag="AM")
            W1 = bp.tile([C1, MEL, MEL], F32, tag="W1")
            W2 = bp.tile([C1, MEL, MEL], F32, tag="W2")
            nc.vector.tensor_tensor(W1[:], bi(x1), bj(x1), op=OP.max)
            nc.vector.tensor_tensor(W2[:], bi(x2), bj(x2), op=OP.min)
            nc.vector.tensor_tensor(W1[:], W2[:], W1[:], op=OP.subtract)
            nc.vector.tensor_scalar(W1[:], W1[:], 0.0, None, op0=OP.max)
            nc.vector.tensor_tensor(AM[:], bi(y1), bj(y1), op=OP.max)
            nc.vector.tensor_tensor(W2[:], bi(y2), bj(y2), op=OP.min)
            nc.vector.tensor_tensor(AM[:], W2[:], AM[:], op=OP.subtract)
            nc.vector.tensor_scalar(AM[:], AM[:], 0.0, None, op0=OP.max)
            nc.vector.tensor_tensor(W1[:], W1[:], AM[:], op=OP.mult)
            nc.vector.tensor_tensor(W2[:], bi(AREA[:]), bj(AREA[:]), op=OP.add)
            nc.vector.tensor_tensor(W2[:], W2[:], W1[:], op=OP.subtract)
            nc.vector.tensor_scalar(W2[:], W2[:], 0.45, 0.45e-8, op0=OP.mult, op1=OP.add)
            nc.vector.tensor_tensor(AM[:], W1[:], W2[:], op=OP.is_gt)
            nc.gpsimd.affine_select(out=AM[:], in_=AM[:], pattern=[[-1, MEL], [1, MEL]],
                                    compare_op=OP.is_ge, fill=0.0, base=-1,
                                    channel_multiplier=0)
            dump("AM", AM[:])

            KEEP = bp.tile([C1, MEL], F32, tag="KEEP")
            nc.vector.memset(KEEP[:], 1.0)
            tk = sm.tile([C1, MEL], F32, tag="tk")
            for i in range(MEL - 1):
                nc.vector.scalar_tensor_tensor(out=tk[:], in0=AM[:, i, :],
                                               scalar=KEEP[:, i:i + 1], in1=KEEP[:],
                                               op0=OP.mult, op1=OP.mult)
                nc.vector.tensor_tensor(KEEP[:], KEEP[:], tk[:], op=OP.subtract)
            dump("KEEP", KEEP[:])

            if past("s7"):
                return
            # ---------- S8: mask, stage merge arrays ----------
            CNDu = sm.tile([C1, MEL], U32, tag="CNDu")
            nc.vector.tensor_scalar(CNDu[:], VAL[:], LN01, None, op0=OP.is_gt)
            KEEPu = sm.tile([C1, MEL], U32, tag="KEEPu")
            nc.vector.tensor_copy(KEEPu[:], KEEP[:])
            nc.vector.tensor_tensor(CNDu[:], CNDu[:], KEEPu[:], op=OP.logical_and)
            NEGT = sm.tile([C1, MEL], F32, tag="NEGT")
            nc.vector.memset(NEGT[:], NEG)
            MSK = bp.tile([C1, MEL], F32, tag="MSK")
            nc.vector.tensor_copy(MSK[:], NEGT[:])
            nc.vector.copy_predicated(MSK[:], CNDu[:], VAL[:])
            dump("MSK", MSK[:])

            E0 = sm.tile([C1, MUSE], U32, tag="E0")
            nc.gpsimd.iota(E0[:], pattern=[[1, MUSE]], base=0, channel_multiplier=MUSE)
            E0F = sm.tile([C1, MUSE], F32, tag="E0F")
            nc.vector.tensor_copy(E0F[:], E0[:])
            CLS1 = sm.tile([C1, 1], U32, tag="CLS1")
            nc.gpsimd.iota(CLS1[:], pattern=[[0, 1]], base=1, channel_multiplier=1)
            AUXT = bp.tile([C1, MUSE, 2], U32, tag="AUXT")
            nc.vector.tensor_copy(AUXT[:, :, 0], ANC[:, :MUSE])
            nc.vector.tensor_copy(AUXT[:, :, 1], CLS1[:].to_broadcast([C1, MUSE]))
            nc.sync.dma_start(AUX.rearrange("(c m) x -> c m x", m=MUSE), AUXT[:])
            # single-writer staging (one DRAM writer per tensor; readback waits on one queue)
            MSKX = bp.tile([94, MUSE], F32, tag="MSKX")
            nc.vector.memset(MSKX[:], NEG)
            nc.vector.tensor_copy(MSKX[0:C1, :], MSK[:, :MUSE])
            nc.sync.dma_start(SK.rearrange("(c m) -> c m", m=MUSE), MSKX[:])
            E0X = bp.tile([94, MUSE], F32, tag="E0X")
            nc.vector.memset(E0X[:], 0.0)
            nc.vector.tensor_copy(E0X[0:C1, :], E0F[:])
            nc.sync.dma_start(SP.rearrange("(c m) -> c m", m=MUSE), E0X[:])

            if past("s8"):
                return
            # ---------- S9: bitonic sort 2048 desc (key cols 0:16, payload cols 16:32) ----------
            KP = bp.tile([P, 32], F32, tag="KP_a")
            nc.sync.dma_start(KP[:, 0:16], SK[0:CAP].rearrange("(p f) -> p f", f=16))
            nc.sync.dma_start(KP[:, 16:32], SP[0:CAP].rearrange("(p f) -> p f", f=16))

            IOTE = bp.tile([P, 16], U32, tag="IOTE")
            nc.gpsimd.iota(IOTE[:], pattern=[[1, 16]], base=0, channel_multiplier=16)
            BITS = []
            for b in range(12):
                bt = bp.tile([P, 16], U32, tag=f"BITS{b}")
                if b == 11:
                    nc.vector.memset(bt[:], 0)
                else:
                    nc.vector.tensor_scalar(bt[:], IOTE[:], b, 1, op0=OP.logical_shift_right,
                                            op1=OP.bitwise_and)
                BITS.append(bt)
            CTu = bp.tile([P, P], U32, tag="CTu")
            nc.gpsimd.iota(CTu[:], pattern=[[1, P]], base=0, channel_multiplier=0)
            RTu = bp.tile([P, P], U32, tag="RTu")
            nc.gpsimd.iota(RTu[:], pattern=[[0, P]], base=0, channel_multiplier=1)
            nc.vector.tensor_tensor(CTu[:], CTu[:], RTu[:], op=OP.bitwise_xor)
            PERMS = []
            for b in range(7):
                pm = bp.tile([P, P], F32, tag=f"PERM{b}")
                pu = sm.tile([P, P], U32, tag="pu")
                nc.vector.tensor_scalar(pu[:], CTu[:], 1 << b, None, op0=OP.is_equal)
                nc.vector.tensor_copy(pm[:], pu[:])
                PERMS.append(pm)

            cur = KP
            other_tag = ["KP_b", "KP_a"]
            flip = 0
            for klog in range(1, 12):
                for jlog in range(klog - 1, -1, -1):
                    wmin = sm.tile([P, 16], U32, tag="wmin")
                    nc.vector.tensor_tensor(wmin[:], BITS[klog][:], BITS[jlog][:],
                                            op=OP.logical_xor)
                    if jlog < 4:
                        j = 1 << jlog
                        pt2 = sm.tile([P, 32], F32, tag="ptF")
                        pv = pt2[:].rearrange("p (h a s l) -> p h a s l", h=2, s=2, l=j)
                        cv = cur[:].rearrange("p (h a s l) -> p h a s l", h=2, s=2, l=j)
                        nc.vector.tensor_copy(pv[:, :, :, 0, :], cv[:, :, :, 1, :])
                        nc.vector.tensor_copy(pv[:, :, :, 1, :], cv[:, :, :, 0, :])
                        ptap = pt2[:]
                    else:
                        pmm = psb.tile([P, 32], F32, tag="ptP", space="PSUM")
                        nc.tensor.matmul(pmm[:], lhsT=PERMS[jlog - 4][:], rhs=cur[:],
                                         start=True, stop=True)
                        ptap = pmm[:]
                    sw = sm.tile([P, 16], U32, tag="sw")
                    nc.vector.tensor_tensor(sw[:], ptap[:, 0:16], cur[:, 0:16], op=OP.is_gt)
                    nc.vector.tensor_tensor(sw[:], sw[:], wmin[:], op=OP.logical_xor)
                    cnd = sm.tile([P, 32], U32, tag="cnd")
                    nc.vector.tensor_copy(cnd[:, 0:16], sw[:])
                    nc.vector.tensor_copy(cnd[:, 16:32], sw[:])
                    nxt = bp.tile([P, 32], F32, tag=other_tag[flip % 2])
                    nc.vector.tensor_copy(nxt[:], cur[:])
                    nc.vector.copy_predicated(nxt[:], cnd[:], ptap)
                    cur = nxt
                    flip += 1
            dump("SORTED", cur[:])

            if past("s9"):
                return
            # ---------- S10: tie repair + partition-parallel output ----------
            nc.sync.dma_start(SOK.rearrange("(p f) -> p f", f=16), cur[0:16, 0:16])
            nc.sync.dma_start(SOP.rearrange("(p f) -> p f", f=16), cur[0:16, 16:32])
            KR = sx.tile([1, 256], F32, tag="KR")
            nc.sync.dma_start(KR[:], SOK.rearrange("(a b) -> a b", a=1))
            PR = sx.tile([1, 256], F32, tag="PR")
            nc.sync.dma_start(PR[:], SOP.rearrange("(a b) -> a b", a=1))
            for par in (0, 1):
                npair = (256 - par) // 2
                kv = KR[:, par:par + 2 * npair].rearrange("a (q two) -> a q two", two=2)
                pv = PR[:, par:par + 2 * npair].rearrange("a (q two) -> a q two", two=2)
                eq = sx.tile([1, npair], U32, tag="eq")
                nc.vector.tensor_tensor(eq[:], kv[:, :, 0], kv[:, :, 1], op=OP.is_equal)
                gt = sx.tile([1, npair], U32, tag="gt")
                nc.vector.tensor_tensor(gt[:], pv[:, :, 0], pv[:, :, 1], op=OP.is_gt)
                nc.vector.tensor_tensor(eq[:], eq[:], gt[:], op=OP.logical_and)
                nl = sx.tile([1, npair], F32, tag="nl")
                nc.vector.tensor_copy(nl[:], pv[:, :, 0])
                nc.vector.copy_predicated(nl[:], eq[:], pv[:, :, 1])
                nr = sx.tile([1, npair], F32, tag="nr")
                nc.vector.tensor_copy(nr[:], pv[:, :, 1])
                nc.vector.copy_predicated(nr[:], eq[:], pv[:, :, 0])
                nc.vector.tensor_copy(pv[:, :, 0], nl[:])
                nc.vector.tensor_copy(pv[:, :, 1], nr[:])
            dump("KR", KR[:])
            dump("PR", PR[:])
            nc.sync.dma_start(SOK2.rearrange("(a b) -> a b", a=1), KR[:])
            nc.sync.dma_start(SOP2.rearrange("(a b) -> a b", a=1), PR[:])

            # relayout rank r = p*2 + f across partitions
            KRB = sx.tile([P, 2], F32, tag="KRB")
            nc.sync.dma_start(KRB[:], SOK2.rearrange("(p f) -> p f", f=2))
            PRB = sx.tile([P, 2], F32, tag="PRB")
            nc.sync.dma_start(PRB[:], SOP2.rearrange("(p f) -> p f", f=2))
            PUB = sx.tile([P, 2], U32, tag="PUB")
            nc.vector.tensor_copy(PUB[:], PRB[:])
            AUXB = sx.tile([P, 2, 2], U32, tag="AUXB")
            for f in range(2):
                nc.gpsimd.indirect_dma_start(
                    out=AUXB[:, f, :], out_offset=None, in_=AUX[:],
                    in_offset=bass.IndirectOffsetOnAxis(ap=PUB[:, f:f + 1], axis=0))
            OBB = sx.tile([P, 2, 4], F32, tag="OBB")
            anc1 = sx.tile([P, 1], U32, tag="anc1")
            for f in range(2):
                nc.vector.tensor_copy(anc1[:], AUXB[:, f, 0:1])
                nc.gpsimd.indirect_dma_start(
                    out=OBB[:, f, :], out_offset=None, in_=BD[:],
                    in_offset=bass.IndirectOffsetOnAxis(ap=anc1[:], axis=0))
            SCR = sx.tile([P, 2], F32, tag="SCR")
            nc.scalar.activation(SCR[:], KRB[:], AF.Exp)
            LBL = sx.tile([P, 2], F32, tag="LBL")
            nc.vector.tensor_copy(LBL[:], AUXB[:, :, 1])
            SRCB = sx.tile([P, 2, 6], F32, tag="SRCB")
            nc.vector.tensor_copy(SRCB[:, :, 0:4], OBB[:])
            nc.vector.tensor_copy(SRCB[:, :, 4], SCR[:])
            nc.vector.tensor_copy(SRCB[:, :, 5], LBL[:])
            nc.sync.dma_start(outs["out"][:].rearrange("(p f) x -> p f x", f=2),
                              SRCB[0:100, :, :])


        for _rep in range(reps):
            emit()


# ======================= host-side runner =======================
import concourse.tile as _tile
import concourse.bacc as _bacc
from concourse import bass_utils as _bass_utils

_CACHE = {}


def _build_nc():
    if "nc" not in _CACHE:
        nc = _bacc.Bacc("TRN2", target_bir_lowering=False, debug=False, num_devices=8)
        ins = {
            "lg": nc.dram_tensor("lg", [NT * P, NC], F32, kind="ExternalInput").ap(),
            "rl": nc.dram_tensor("rl", [NT * P, 4], F32, kind="ExternalInput").ap(),
            "an": nc.dram_tensor("an", [NT * P, 4], F32, kind="ExternalInput").ap(),
        }
        outs = {"out": nc.dram_tensor("out", [OUTN, 6], F32, kind="ExternalOutput").ap()}
        with _tile.TileContext(nc) as tc:
            build_kernel(tc, outs, ins)
        nc.compile()
        _CACHE["nc"] = nc
    return _CACHE["nc"]


def _pad_image(logits, rel, anchors_pad):
    NPAD = NT * P
    L = np.zeros((NPAD, NC), np.float32); L[:8732] = logits
    R = np.zeros((NPAD, 4), np.float32); R[:8732] = rel
    return {"lg": L, "rl": R, "an": anchors_pad}


def _run(bbox_regression, cls_logits, anchors, trace=False):
    nc = _build_nc()
    NPAD = NT * P
    A = np.tile(np.array([0, 0, 1, 1], np.float32), (NPAD, 1))
    A[:8732] = anchors
    B = cls_logits.shape[0]
    in_maps = [_pad_image(cls_logits[b], bbox_regression[b], A) for b in range(B)]
    res = _bass_utils.run_bass_kernel_spmd(nc, in_maps, core_ids=list(range(B)),
                                           trace=trace)
    out = np.stack([res.results[b]["out"] for b in range(B)]).astype(np.float32)
    return out, res


def kernel(bbox_regression, cls_logits, anchors):
    out, _ = _run(np.asarray(bbox_regression), np.asarray(cls_logits),
                  np.asarray(anchors))
    return out



# revision 3
# speedup vs baseline: 376.8566x; 376.8566x over previous
"""Optimized Bass/Tile kernel body for SSD postprocess (one image per core).

Sort keys are softmax scores (no log space). Gathers are class-major direct:
offsets live in [90,16] SBUF tiles, one indirect DMA per column, outputs land
class-major (no DRAM relayout roundtrips). Elementwise work is split between
DVE and GPSIMD(Pool); PSUM->SBUF copies ride Act/Pool; sort runs on 1024
entries with precomputed direction masks and in-place predicated updates.

Emission order (= queue order) is tuned for overlap: setup first (overlaps
input DMA), box decode is emitted after the value-gather issue so it fills
the DVE bubble during gather latency.
"""
import numpy as np
import concourse.bass as bass
import concourse.mybir as mybir
from concourse.masks import make_identity

P = 128
NT = 69            # anchor tiles (8832 = 69*128)
NC = 91
C1 = 90
NANC = NT * P      # 8832
NCH = NANC // 8    # 1104 chunks of 8 consecutive anchors per class
MCH = 12           # top chunks per class
MEL = 12           # top elements per class
MUSE = 11          # columns merged globally (90*11 = 990)
CAP = 1024
CLOG = 10          # log2(CAP)
FW = 8             # sort cols per partition (CAP = 128*FW)
OUTN = 200
NEG = -1e30
SCORE_THRESH = 0.01
BBOX_CLIP = float(np.log(1000.0 / 16.0))

F32 = mybir.dt.float32
U32 = mybir.dt.uint32
OP = mybir.AluOpType
AF = mybir.ActivationFunctionType
AX = mybir.AxisListType

STAGES = ["s1", "s2b", "s3", "s4", "s5", "s2", "s6", "s7", "s8", "s9", "s10a", "s10b", "s10c", "s10d", "s10e", "s10f", "full"]


def build_kernel(tc, outs, ins, dbg=None, stop_after="full", reps=1):
    nc = tc.nc
    dbg = dbg or {}
    LIMIT = STAGES.index(stop_after)

    def past(stage):
        return STAGES.index(stage) > LIMIT

    def dump(name, ap):
        if name in dbg:
            nc.sync.dma_start(dbg[name][:], ap)

    # DRAM scratch (offset-0 tensors; indirect DMA requires offset==0)
    TDC = nc.dram_tensor("tdc_scratch", [C1 * NCH, 8], F32, kind="Internal").ap()
    BD = nc.dram_tensor("bd_scratch", [NANC, 4], F32, kind="Internal").ap()
    CMB = nc.dram_tensor("cmb_scratch", [C1 * MUSE, 8], F32, kind="Internal").ap()
    SK = nc.dram_tensor("sk_scratch", [94 * MUSE], F32, kind="Internal").ap()
    SPd = nc.dram_tensor("sp_scratch", [94 * MUSE], F32, kind="Internal").ap()

    with tc.tile_pool(name="big", bufs=1) as bp, \
         tc.tile_pool(name="sm", bufs=2) as sm, \
         tc.tile_pool(name="sx", bufs=1) as sx, \
         tc.tile_pool(name="ps", bufs=2, space="PSUM") as ps, \
         tc.tile_pool(name="psb", bufs=4, space="PSUM") as psb:

        def emit():
            # ================= S0: input-independent setup =================
            ident = bp.tile([P, P], F32, tag="ident")
            make_identity(nc, ident[:])
            IOTE = bp.tile([P, FW], U32, tag="IOTE")
            nc.gpsimd.iota(IOTE[:], pattern=[[1, FW]], base=0, channel_multiplier=FW)
            BITS = []
            for b in range(CLOG + 1):
                bt = bp.tile([P, FW], U32, tag=f"BITS{b}")
                if b == CLOG:
                    nc.vector.memset(bt[:], 0)
                else:
                    nc.vector.tensor_scalar(bt[:], IOTE[:], b, 1,
                                            op0=OP.logical_shift_right,
                                            op1=OP.bitwise_and)
                BITS.append(bt)
            WMIN = {}
            for klog in range(1, CLOG + 1):
                for jlog in range(klog - 1, -1, -1):
                    wt = bp.tile([P, FW], U32, tag=f"WM{klog}_{jlog}")
                    nc.vector.tensor_tensor(wt[:], BITS[klog][:], BITS[jlog][:],
                                            op=OP.logical_xor)
                    WMIN[(klog, jlog)] = wt
            CTu = bp.tile([P, P], U32, tag="CTu")
            nc.gpsimd.iota(CTu[:], pattern=[[1, P]], base=0, channel_multiplier=0)
            RTu = bp.tile([P, P], U32, tag="RTu")
            nc.gpsimd.iota(RTu[:], pattern=[[0, P]], base=0, channel_multiplier=1)
            nc.vector.tensor_tensor(CTu[:], CTu[:], RTu[:], op=OP.bitwise_xor)
            PERMS = []
            for b in range(7):
                pm = bp.tile([P, P], F32, tag=f"PERM{b}")
                pu = sm.tile([P, P], U32, tag="pu")
                nc.vector.tensor_scalar(pu[:], CTu[:], 1 << b, None, op0=OP.is_equal)
                nc.vector.tensor_copy(pm[:], pu[:])
                PERMS.append(pm)
            # shift matrices for the S10 cross-partition tie repair
            CT2 = bp.tile([P, P], U32, tag="CT2")
            nc.gpsimd.iota(CT2[:], pattern=[[1, P]], base=0, channel_multiplier=0)
            shm = sm.tile([P, P], U32, tag="shm")
            nc.vector.tensor_scalar(shm[:], CT2[:], 1, None, op0=OP.add)
            shu = sm.tile([P, P], U32, tag="shu")
            nc.vector.tensor_tensor(shu[:], RTu[:], shm[:], op=OP.is_equal)
            SU = bp.tile([P, P], F32, tag="SU")
            nc.vector.tensor_copy(SU[:], shu[:])
            nc.vector.tensor_scalar(shm[:], RTu[:], 1, None, op0=OP.add)
            nc.vector.tensor_tensor(shu[:], shm[:], CT2[:], op=OP.is_equal)
            SD = bp.tile([P, P], F32, tag="SD")
            nc.vector.tensor_copy(SD[:], shu[:])
            CBASE = bp.tile([C1, 1], U32, tag="CBASE")
            nc.gpsimd.iota(CBASE[:], pattern=[[0, 1]], base=0, channel_multiplier=NCH)
            KI = bp.tile([C1, MCH], U32, tag="KI")
            nc.gpsimd.iota(KI[:], pattern=[[1, MCH]], base=0, channel_multiplier=0)
            KIF = bp.tile([C1, MCH], F32, tag="KIF")
            nc.vector.tensor_copy(KIF[:], KI[:])
            E0 = bp.tile([C1, MUSE], U32, tag="E0")
            nc.gpsimd.iota(E0[:], pattern=[[1, MUSE]], base=0, channel_multiplier=MUSE)
            E0F = bp.tile([C1, MUSE], F32, tag="E0F")
            nc.vector.tensor_copy(E0F[:], E0[:])
            CLS1 = bp.tile([C1, 1], U32, tag="CLS1")
            nc.gpsimd.iota(CLS1[:], pattern=[[0, 1]], base=1, channel_multiplier=1)
            MSKX = bp.tile([94, MUSE], F32, tag="MSKX")
            nc.vector.memset(MSKX[:], NEG)
            E0X = bp.tile([94, MUSE], F32, tag="E0X")
            nc.vector.memset(E0X[:], 0.0)
            nc.vector.tensor_copy(E0X[0:C1, :], E0F[:])

            # ====== S1+S2b: 3-chunk softmax -> transpose -> evac pipeline =====
            # Act's LG slice is issued first on its queue so exp starts the
            # moment it lands; payload staging rides sync instead.
            LG = bp.tile([P, NT, NC], F32, tag="LG")
            lgsrc = ins["lg"][:].rearrange("(t p) c -> p t c", p=P)
            CHS = [(0, 24), (24, 48), (48, NT)]
            nc.scalar.dma_start(LG[:, 0:24, :], lgsrc[:, 0:24, :])
            nc.sync.dma_start(LG[:, 24:48, :], lgsrc[:, 24:48, :])
            nc.gpsimd.dma_start(LG[:, 48:NT, :], lgsrc[:, 48:NT, :])
            RL = bp.tile([P, NT, 4], F32, tag="RL")
            nc.sync.dma_start(RL[:], ins["rl"][:].rearrange("(t p) c -> p t c", p=P))
            AN = bp.tile([P, NT, 4], F32, tag="AN")
            nc.gpsimd.dma_start(AN[:], ins["an"][:].rearrange("(t p) c -> p t c", p=P))
            nc.sync.dma_start(SPd.rearrange("(c m) -> c m", m=MUSE), E0X[:])

            E = bp.tile([P, NT, NC], F32, tag="E")
            ZT1 = bp.tile([P, NT, 46], F32, tag="ZT1")
            ZT2 = bp.tile([P, NT, 23], F32, tag="ZT2")
            Z = sm.tile([P, NT], F32, tag="Z")
            RZ = sm.tile([P, NT], F32, tag="RZ")
            TCM = bp.tile([C1, NANC], F32, tag="TCM")  # classes 1..90 on parts 0..89
            tdc = TDC.rearrange("(c h) i -> c (h i)", c=C1)
            gidx = 0
            for ci, (ta, tb) in enumerate(CHS):
                w = tb - ta
                nc.scalar.activation(E[:, ta:tb, :], LG[:, ta:tb, :], AF.Exp)
                eng = nc.vector if ci % 2 == 0 else nc.gpsimd
                eng.tensor_tensor(ZT1[:, ta:tb, 0:45], E[:, ta:tb, 0:45],
                                  E[:, ta:tb, 46:91], op=OP.add)
                eng.tensor_copy(ZT1[:, ta:tb, 45], E[:, ta:tb, 45])
                eng.tensor_tensor(ZT2[:, ta:tb, :], ZT1[:, ta:tb, 0:23],
                                  ZT1[:, ta:tb, 23:46], op=OP.add)
                if ci % 2 == 0:
                    nc.vector.tensor_reduce(Z[:, ta:tb], ZT2[:, ta:tb, :],
                                            axis=AX.X, op=OP.add)
                    nc.vector.reciprocal(RZ[:, ta:tb], Z[:, ta:tb])
                    nc.vector.tensor_tensor(
                        E[:, ta:tb, :], E[:, ta:tb, :],
                        RZ[:, ta:tb].to_broadcast([P, w, NC]), op=OP.mult)
                else:
                    # Pool lacks free-axis reduce: finish the tree pairwise
                    ZT3 = sm.tile([P, NT, 12], F32, tag="ZT3")
                    nc.gpsimd.tensor_tensor(ZT3[:, ta:tb, 0:11], ZT2[:, ta:tb, 0:11],
                                            ZT2[:, ta:tb, 12:23], op=OP.add)
                    nc.gpsimd.tensor_copy(ZT3[:, ta:tb, 11], ZT2[:, ta:tb, 11])
                    ZT4 = sm.tile([P, NT, 6], F32, tag="ZT4")
                    nc.gpsimd.tensor_tensor(ZT4[:, ta:tb, :], ZT3[:, ta:tb, 0:6],
                                            ZT3[:, ta:tb, 6:12], op=OP.add)
                    ZT5 = sm.tile([P, NT, 3], F32, tag="ZT5")
                    nc.gpsimd.tensor_tensor(ZT5[:, ta:tb, :], ZT4[:, ta:tb, 0:3],
                                            ZT4[:, ta:tb, 3:6], op=OP.add)
                    nc.gpsimd.tensor_tensor(Z[:, ta:tb], ZT5[:, ta:tb, 0],
                                            ZT5[:, ta:tb, 1], op=OP.add)
                    nc.gpsimd.tensor_tensor(Z[:, ta:tb], Z[:, ta:tb],
                                            ZT5[:, ta:tb, 2], op=OP.add)
                    nc.vector.reciprocal(RZ[:, ta:tb], Z[:, ta:tb])
                    nc.gpsimd.tensor_tensor(
                        E[:, ta:tb, :], E[:, ta:tb, :],
                        RZ[:, ta:tb].to_broadcast([P, w, NC]), op=OP.mult)
                for t0 in range(ta, tb, 4):
                    n = min(4, tb - t0)
                    pt = ps.tile([C1, 4, P], F32, tag="trp", space="PSUM")
                    for j in range(n):
                        nc.tensor.transpose(out=pt[:, j, :], in_=E[:, t0 + j, 1:NC],
                                            identity=ident[:])
                    if gidx % 2 == 0:
                        nc.scalar.copy(TCM[:, t0 * P:(t0 + n) * P],
                                       pt[:, 0:n, :].rearrange("c a b -> c (a b)"))
                    else:
                        nc.vector.tensor_copy(
                            TCM[:, t0 * P:(t0 + n) * P],
                            pt[:, 0:n, :].rearrange("c a b -> c (a b)"))
                    gidx += 1
                q = [nc.sync, nc.gpsimd, nc.scalar][ci]
                q.dma_start(tdc[:, ta * P:tb * P], TCM[:, ta * P:tb * P])
            dump("T", E[:])
            dump("TCM", TCM[:])

            if past("s1"):
                return
            if past("s2b"):
                return
            # ====== S3: chunk max via pairwise-max tree (DVE; Pool lacks
            # tensor_tensor max) — 3 halving levels beat one 8-wide reduce ====
            t4 = TCM[:].rearrange("c (h two) -> c h two", two=2)   # [C1, 4416, 2]
            L1 = bp.tile([C1, NCH * 4], F32, tag="L1")
            nc.vector.tensor_tensor(L1[:], t4[:, :, 0], t4[:, :, 1], op=OP.max)
            l4 = L1[:].rearrange("c (h two) -> c h two", two=2)    # [C1, 2208, 2]
            L2 = bp.tile([C1, NCH * 2], F32, tag="L2")
            nc.vector.tensor_tensor(L2[:], l4[:, :, 0], l4[:, :, 1], op=OP.max)
            l5 = L2[:].rearrange("c (h two) -> c h two", two=2)    # [C1, 1104, 2]
            CC = bp.tile([C1, NCH], F32, tag="CC")
            nc.vector.tensor_tensor(CC[:], l5[:, :, 0], l5[:, :, 1], op=OP.max)
            dump("CC", CC[:])

            if past("s3"):
                return
            # ================= S4: top-16 chunks per class ====================
            CHI = bp.tile([C1, MCH], U32, tag="CHI")
            CHIF = bp.tile([C1, MCH], F32, tag="CHIF")
            OFF1 = bp.tile([C1, MCH], U32, tag="OFF1")
            GV = bp.tile([C1, MCH, 8], F32, tag="GV")
            for r in range(2):
                lo, hi = 8 * r, min(8 * r + 8, MCH)
                m8 = sm.tile([C1, 8], F32, tag="m8")
                nc.vector.max(out=m8[:], in_=CC[:])
                i8 = sm.tile([C1, 8], U32, tag="i8")
                nc.vector.max_index(out=i8[:], in_max=m8[:], in_values=CC[:])
                nc.vector.tensor_copy(CHI[:, lo:hi], i8[:, 0:hi - lo])
                if r < 1:
                    nc.vector.match_replace(out=CC[:], in_to_replace=m8[:],
                                            in_values=CC[:], imm_value=NEG)
                # launch this round's gathers immediately (overlaps next round)
                nc.vector.tensor_tensor(OFF1[:, lo:hi], CHI[:, lo:hi],
                                        CBASE[:].to_broadcast([C1, hi - lo]),
                                        op=OP.add)
                for t in range(lo, hi):
                    nc.gpsimd.indirect_dma_start(
                        out=GV[:, t, :], out_offset=None, in_=TDC[:],
                        in_offset=bass.IndirectOffsetOnAxis(ap=OFF1[:, t:t + 1],
                                                            axis=0))
            nc.vector.tensor_copy(CHIF[:], CHI[:])
            dump("CHI", CHI[:])

            if past("s4"):
                return
            # ========== S5: gathers launched above (class-major direct) =======
            dump("GV", GV[:].rearrange("c k i -> c (k i)"))

            if past("s5"):
                return
            # ===== S2: decode boxes (emitted here to fill the gather bubble) ==
            def col(t, k):
                return t[:, :, k]

            BX = bp.tile([P, NT, 4], F32, tag="BX")
            aw = sm.tile([P, NT], F32, tag="aw")
            nc.vector.tensor_tensor(aw[:], col(AN, 2), col(AN, 0), op=OP.subtract)
            ah = sm.tile([P, NT], F32, tag="ah")
            nc.gpsimd.tensor_tensor(ah[:], col(AN, 3), col(AN, 1), op=OP.subtract)
            ax = sm.tile([P, NT], F32, tag="ax")
            nc.vector.tensor_scalar(ax[:], aw[:], 0.5, None, op0=OP.mult)
            nc.vector.tensor_tensor(ax[:], ax[:], col(AN, 0), op=OP.add)
            ay = sm.tile([P, NT], F32, tag="ay")
            nc.gpsimd.tensor_scalar(ay[:], ah[:], 0.5, None, op0=OP.mult)
            nc.gpsimd.tensor_tensor(ay[:], ay[:], col(AN, 1), op=OP.add)

            dx = sm.tile([P, NT], F32, tag="dx")
            nc.vector.tensor_scalar(dx[:], col(RL, 0), 0.1, None, op0=OP.mult)
            px = sm.tile([P, NT], F32, tag="px")
            nc.vector.tensor_tensor(px[:], dx[:], aw[:], op=OP.mult)
            nc.vector.tensor_tensor(px[:], px[:], ax[:], op=OP.add)
            dw = sm.tile([P, NT], F32, tag="dw")
            nc.vector.tensor_scalar(dw[:], col(RL, 2), 0.2, BBOX_CLIP, op0=OP.mult,
                                    op1=OP.min)
            ew = sm.tile([P, NT], F32, tag="ew")
            nc.scalar.activation(ew[:], dw[:], AF.Exp)
            pw = sm.tile([P, NT], F32, tag="pw")
            nc.vector.tensor_tensor(pw[:], ew[:], aw[:], op=OP.mult)
            nc.vector.tensor_scalar(pw[:], pw[:], 0.5, None, op0=OP.mult)

            dy = sm.tile([P, NT], F32, tag="dy")
            nc.gpsimd.tensor_scalar(dy[:], col(RL, 1), 0.1, None, op0=OP.mult)
            py = sm.tile([P, NT], F32, tag="py")
            nc.gpsimd.tensor_tensor(py[:], dy[:], ah[:], op=OP.mult)
            nc.gpsimd.tensor_tensor(py[:], py[:], ay[:], op=OP.add)
            dh = sm.tile([P, NT], F32, tag="dh")
            nc.gpsimd.tensor_scalar(dh[:], col(RL, 3), 0.2, BBOX_CLIP, op0=OP.mult,
                                    op1=OP.min)
            eh = sm.tile([P, NT], F32, tag="eh")
            nc.scalar.activation(eh[:], dh[:], AF.Exp)
            ph = sm.tile([P, NT], F32, tag="ph")
            nc.gpsimd.tensor_tensor(ph[:], eh[:], ah[:], op=OP.mult)
            nc.gpsimd.tensor_scalar(ph[:], ph[:], 0.5, None, op0=OP.mult)

            tq = sm.tile([P, NT], F32, tag="tq")
            nc.vector.tensor_tensor(tq[:], px[:], pw[:], op=OP.subtract)
            nc.vector.tensor_scalar(col(BX, 0), tq[:], 0.0, 300.0, op0=OP.max,
                                    op1=OP.min)
            nc.vector.tensor_tensor(tq[:], px[:], pw[:], op=OP.add)
            nc.vector.tensor_scalar(col(BX, 2), tq[:], 0.0, 300.0, op0=OP.max,
                                    op1=OP.min)
            tq2 = sm.tile([P, NT], F32, tag="tq2")
            nc.gpsimd.tensor_tensor(tq2[:], py[:], ph[:], op=OP.subtract)
            nc.gpsimd.tensor_scalar(col(BX, 1), tq2[:], 0.0, 300.0, op0=OP.max,
                                    op1=OP.min)
            nc.gpsimd.tensor_tensor(tq2[:], py[:], ph[:], op=OP.add)
            nc.gpsimd.tensor_scalar(col(BX, 3), tq2[:], 0.0, 300.0, op0=OP.max,
                                    op1=OP.min)
            nc.sync.dma_start(BD.rearrange("(t p) c -> p t c", p=P), BX[:])
            dump("BX", BX[:])

            if past("s2"):
                return
            # ========= S6: top-16 elements; anchors via index formula =========
            GVf = GV[:].rearrange("c k i -> c (k i)")
            GVK = bp.tile([C1, MCH * 8], F32, tag="GVK")
            nc.vector.tensor_copy(GVK[:], GVf)
            VAL = bp.tile([C1, MEL], F32, tag="VAL")
            IDX = bp.tile([C1, MEL], U32, tag="IDX")
            for r in range(2):
                m8b = sm.tile([C1, 8], F32, tag="m8b")
                nc.vector.max(out=m8b[:], in_=GVK[:])
                i8b = sm.tile([C1, 8], U32, tag="i8b")
                nc.vector.max_index(out=i8b[:], in_max=m8b[:], in_values=GVK[:])
                w = min(8, MEL - 8 * r)
                nc.vector.tensor_copy(VAL[:, 8 * r:8 * r + w], m8b[:, 0:w])
                nc.vector.tensor_copy(IDX[:, 8 * r:8 * r + w], i8b[:, 0:w])
                if r < 1:
                    nc.vector.match_replace(out=GVK[:], in_to_replace=m8b[:],
                                            in_values=GVK[:], imm_value=NEG)
            dump("VAL", VAL[:])
            KHI = sm.tile([C1, MEL], U32, tag="KHI")
            nc.vector.tensor_scalar(KHI[:], IDX[:], 3, None, op0=OP.logical_shift_right)
            KLO = sm.tile([C1, MEL], U32, tag="KLO")
            nc.vector.tensor_scalar(KLO[:], IDX[:], 7, None, op0=OP.bitwise_and)
            KLOF = sm.tile([C1, MEL], F32, tag="KLOF")
            nc.vector.tensor_copy(KLOF[:], KLO[:])
            KHIF = sm.tile([C1, MEL], F32, tag="KHIF")
            nc.vector.tensor_copy(KHIF[:], KHI[:])
            EQ = sx.tile([C1, MEL, MCH], F32, tag="EQ")
            nc.vector.tensor_tensor(EQ[:], KHIF[:].to_broadcast([C1, MEL, MCH]),
                                    KIF[:].to_broadcast([C1, MCH, MEL]).rearrange(
                                        "c a b -> c b a"), op=OP.is_equal)
            nc.vector.tensor_tensor(EQ[:], EQ[:],
                                    CHIF[:].to_broadcast([C1, MCH, MEL]).rearrange(
                                        "c a b -> c b a"), op=OP.mult)
            ANCF = bp.tile([C1, MEL], F32, tag="ANCF")
            nc.vector.tensor_reduce(ANCF[:], EQ[:], axis=AX.X, op=OP.add)
            nc.vector.tensor_scalar(ANCF[:], ANCF[:], 8.0, None, op0=OP.mult)
            nc.vector.tensor_tensor(ANCF[:], ANCF[:], KLOF[:], op=OP.add)
            ANC = bp.tile([C1, MEL], U32, tag="ANC")
            nc.vector.tensor_copy(ANC[:], ANCF[:])
            dump("ANC", ANC[:])
            dump("IDX", IDX[:])
            dump("ANCF", ANCF[:])

            if past("s6"):
                return
            # ========= S7: gather boxes (class-major direct), greedy NMS ======
            BOXG = bp.tile([C1, MEL, 4], F32, tag="BOXG")
            for t in range(MEL):
                nc.gpsimd.indirect_dma_start(
                    out=BOXG[:, t, :], out_offset=None, in_=BD[:],
                    in_offset=bass.IndirectOffsetOnAxis(ap=ANC[:, t:t + 1], axis=0))
            x1 = BOXG[:, :, 0]; y1 = BOXG[:, :, 1]; x2 = BOXG[:, :, 2]; y2 = BOXG[:, :, 3]
            AREA = sm.tile([C1, MEL], F32, tag="AREA")
            wq = sm.tile([C1, MEL], F32, tag="wq")
            nc.vector.tensor_tensor(wq[:], x2, x1, op=OP.subtract)
            nc.vector.tensor_tensor(AREA[:], y2, y1, op=OP.subtract)
            nc.vector.tensor_tensor(AREA[:], AREA[:], wq[:], op=OP.mult)

            def bi(apv):
                return apv.to_broadcast([C1, MEL, MEL])

            def bj(apv):
                return apv.to_broadcast([C1, MEL, MEL]).rearrange("c a b -> c b a")

            AM = bp.tile([C1, MEL, MEL], F32, tag="AM")
            W1 = bp.tile([C1, MEL, MEL], F32, tag="W1")
            W2 = bp.tile([C1, MEL, MEL], F32, tag="W2")
            nc.vector.tensor_tensor(W1[:], bi(x1), bj(x1), op=OP.max)
            nc.vector.tensor_tensor(W2[:], bi(x2), bj(x2), op=OP.min)
            nc.vector.tensor_tensor(W1[:], W2[:], W1[:], op=OP.subtract)
            nc.vector.tensor_scalar(W1[:], W1[:], 0.0, None, op0=OP.max)
            nc.vector.tensor_tensor(AM[:], bi(y1), bj(y1), op=OP.max)
            nc.vector.tensor_tensor(W2[:], bi(y2), bj(y2), op=OP.min)
            nc.vector.tensor_tensor(AM[:], W2[:], AM[:], op=OP.subtract)
            nc.vector.tensor_scalar(AM[:], AM[:], 0.0, None, op0=OP.max)
            nc.vector.tensor_tensor(W1[:], W1[:], AM[:], op=OP.mult)
            nc.vector.tensor_tensor(W2[:], bi(AREA[:]), bj(AREA[:]), op=OP.add)
            nc.vector.tensor_tensor(W2[:], W2[:], W1[:], op=OP.subtract)
            nc.vector.tensor_scalar(W2[:], W2[:], 0.45, 0.45e-8, op0=OP.mult,
                                    op1=OP.add)
            nc.vector.tensor_tensor(AM[:], W1[:], W2[:], op=OP.is_gt)
            nc.gpsimd.affine_select(out=AM[:], in_=AM[:], pattern=[[-1, MEL], [1, MEL]],
                                    compare_op=OP.is_ge, fill=0.0, base=-1,
                                    channel_multiplier=0)
            dump("AM", AM[:])

            KEEP = bp.tile([C1, MEL], F32, tag="KEEP")
            nc.vector.memset(KEEP[:], 1.0)
            tk = sm.tile([C1, MEL], F32, tag="tk")
            for i in range(MEL - 1):
                nc.vector.scalar_tensor_tensor(out=tk[:], in0=AM[:, i, :],
                                               scalar=KEEP[:, i:i + 1], in1=KEEP[:],
                                               op0=OP.mult, op1=OP.mult)
                nc.vector.tensor_tensor(KEEP[:], KEEP[:], tk[:], op=OP.subtract)
            dump("KEEP", KEEP[:])

            if past("s7"):
                return
            # ================= S8: mask, stage merge arrays ===================
            CNDu = sm.tile([C1, MEL], U32, tag="CNDu")
            nc.vector.tensor_scalar(CNDu[:], VAL[:], SCORE_THRESH, None, op0=OP.is_gt)
            KEEPu = sm.tile([C1, MEL], U32, tag="KEEPu")
            nc.vector.tensor_copy(KEEPu[:], KEEP[:])
            nc.vector.tensor_tensor(CNDu[:], CNDu[:], KEEPu[:], op=OP.logical_and)
            MSK = bp.tile([C1, MEL], F32, tag="MSK")
            nc.vector.memset(MSK[:], NEG)
            nc.vector.copy_predicated(MSK[:], CNDu[:], VAL[:])
            dump("MSK", MSK[:])

            # combined per-candidate record: box(4) + label(1); one gather in S10
            CMBT = bp.tile([C1, MUSE, 8], F32, tag="CMBT")
            nc.gpsimd.memset(CMBT[:].rearrange("c m x -> c (m x)"), 0.0)
            nc.gpsimd.tensor_copy(CMBT[:, :, 0:4], BOXG[:, 0:MUSE, :])
            CLS1F = sm.tile([C1, 1], F32, tag="CLS1F")
            nc.gpsimd.tensor_copy(CLS1F[:], CLS1[:])
            nc.gpsimd.tensor_copy(CMBT[:, :, 4], CLS1F[:].to_broadcast([C1, MUSE]))
            nc.gpsimd.dma_start(CMB.rearrange("(c m) x -> c m x", m=MUSE), CMBT[:])
            nc.vector.tensor_copy(MSKX[0:C1, :], MSK[:, :MUSE])
            nc.sync.dma_start(SK.rearrange("(c m) -> c m", m=MUSE), MSKX[:])

            if past("s8"):
                return
            # ===== S9: bitonic sort 1024 desc; keys/payload in separate ====
            # tiles so stage t+1's key permute (PE) overlaps stage t's payload
            # update (DVE)
            KPK = bp.tile([P, FW], F32, tag="KPK")
            KPP = bp.tile([P, FW], F32, tag="KPP")
            nc.sync.dma_start(KPK[:], SK[0:CAP].rearrange("(p f) -> p f", f=FW))
            nc.scalar.dma_start(KPP[:], SPd[0:CAP].rearrange("(p f) -> p f", f=FW))
            for klog in range(1, CLOG + 1):
                for jlog in range(klog - 1, -1, -1):
                    wmin = WMIN[(klog, jlog)]
                    pmk = psb.tile([P, FW], F32, tag="pmk", space="PSUM")
                    pmp = psb.tile([P, FW], F32, tag="pmp", space="PSUM")
                    if jlog < 3:
                        j = 1 << jlog
                        kv = KPK[:].rearrange("p (a s l) -> p a s l", s=2, l=j)
                        pv = KPP[:].rearrange("p (a s l) -> p a s l", s=2, l=j)
                        nc.tensor.matmul(
                            pmk[:].rearrange("p (a s l) -> p a s l", s=2, l=j),
                            lhsT=ident[:], rhs=kv[:, :, ::-1, :],
                            start=True, stop=True)
                        nc.tensor.matmul(
                            pmp[:].rearrange("p (a s l) -> p a s l", s=2, l=j),
                            lhsT=ident[:], rhs=pv[:, :, ::-1, :],
                            start=True, stop=True)
                    else:
                        nc.tensor.matmul(pmk[:], lhsT=PERMS[jlog - 3][:], rhs=KPK[:],
                                         start=True, stop=True)
                        nc.tensor.matmul(pmp[:], lhsT=PERMS[jlog - 3][:], rhs=KPP[:],
                                         start=True, stop=True)
                    sw = sm.tile([P, FW], U32, tag="sw")
                    nc.vector.tensor_tensor(sw[:], pmk[:], KPK[:], op=OP.is_gt)
                    nc.vector.tensor_tensor(sw[:], sw[:], wmin[:], op=OP.logical_xor)
                    nc.vector.copy_predicated(KPK[:], sw[:], pmk[:])
                    nc.vector.copy_predicated(KPP[:], sw[:], pmp[:])
            dump("SORTEDK", KPK[:])
            dump("SORTEDP", KPP[:])

            if past("s9"):
                return
            # ========== S10: tie repair in [32,8], direct output ==============
            K32 = KPK[0:32, :]
            P32 = KPP[0:32, :]
            kvA = K32.rearrange("p (q two) -> p q two", two=2)
            pvA = P32.rearrange("p (q two) -> p q two", two=2)
            eqA = sx.tile([32, 4], U32, tag="eqA")
            nc.vector.tensor_tensor(eqA[:], kvA[:, :, 0], kvA[:, :, 1], op=OP.is_equal)
            gtA = sx.tile([32, 4], U32, tag="gtA")
            nc.vector.tensor_tensor(gtA[:], pvA[:, :, 0], pvA[:, :, 1], op=OP.is_gt)
            nc.vector.tensor_tensor(eqA[:], eqA[:], gtA[:], op=OP.logical_and)
            swpA = sx.tile([32, 4, 2], F32, tag="swpA")
            nc.vector.tensor_copy(swpA[:, :, 0], pvA[:, :, 1])
            nc.vector.tensor_copy(swpA[:, :, 1], pvA[:, :, 0])
            cndA = sx.tile([32, 4, 2], U32, tag="cndA")
            nc.vector.tensor_copy(cndA[:, :, 0], eqA[:])
            nc.vector.tensor_copy(cndA[:, :, 1], eqA[:])
            nc.vector.copy_predicated(pvA[:], cndA[:], swpA[:])
            kvB = K32[:, 1:7].rearrange("p (q two) -> p q two", two=2)
            pvB = P32[:, 1:7].rearrange("p (q two) -> p q two", two=2)
            eqB = sx.tile([32, 3], U32, tag="eqB")
            nc.vector.tensor_tensor(eqB[:], kvB[:, :, 0], kvB[:, :, 1], op=OP.is_equal)
            gtB = sx.tile([32, 3], U32, tag="gtB")
            nc.vector.tensor_tensor(gtB[:], pvB[:, :, 0], pvB[:, :, 1], op=OP.is_gt)
            nc.vector.tensor_tensor(eqB[:], eqB[:], gtB[:], op=OP.logical_and)
            swpB = sx.tile([32, 3, 2], F32, tag="swpB")
            nc.vector.tensor_copy(swpB[:, :, 0], pvB[:, :, 1])
            nc.vector.tensor_copy(swpB[:, :, 1], pvB[:, :, 0])
            cndB = sx.tile([32, 3, 2], U32, tag="cndB")
            nc.vector.tensor_copy(cndB[:, :, 0], eqB[:])
            nc.vector.tensor_copy(cndB[:, :, 1], eqB[:])
            nc.vector.copy_predicated(pvB[:], cndB[:], swpB[:])
            if past("s10a"):
                return
            # parity B boundary: (p,7) vs (p+1,0) via PE shift (no DRAM trip)
            pshU = psb.tile([P, FW], F32, tag="pmk", space="PSUM")
            nc.tensor.matmul(pshU[:, 0:1], lhsT=SU[:], rhs=KPK[:, 0:1],
                             start=True, stop=True)
            nc.tensor.matmul(pshU[:, 1:2], lhsT=SU[:], rhs=KPP[:, 0:1],
                             start=True, stop=True)
            eqC = sx.tile([32, 1], U32, tag="eqC")
            nc.vector.tensor_tensor(eqC[:], K32[:, 7:8], pshU[0:32, 0:1],
                                    op=OP.is_equal)
            gtC = sx.tile([32, 1], U32, tag="gtC")
            nc.vector.tensor_tensor(gtC[:], P32[:, 7:8], pshU[0:32, 1:2],
                                    op=OP.is_gt)
            nc.vector.tensor_tensor(eqC[:], eqC[:], gtC[:], op=OP.logical_and)
            PB7 = sx.tile([P, 2], F32, tag="PB7")
            nc.vector.memset(PB7[:], 0.0)
            nc.vector.tensor_copy(PB7[0:32, 0:1], P32[:, 7:8])
            nc.vector.tensor_copy(PB7[0:32, 1:2], eqC[:])
            nc.vector.copy_predicated(P32[:, 7:8], eqC[:], pshU[0:32, 1:2])
            pshD = psb.tile([P, FW], F32, tag="pmp", space="PSUM")
            nc.tensor.matmul(pshD[:, 0:2], lhsT=SD[:], rhs=PB7[:], start=True,
                             stop=True)
            eq2 = sx.tile([32, 1], U32, tag="eq2")
            nc.vector.tensor_copy(eq2[:], pshD[0:32, 1:2])
            nc.vector.copy_predicated(P32[:, 0:1], eq2[:], pshD[0:32, 0:1])
            dump("KR", K32)
            dump("PR", P32)
            if past("s10b"):
                return

            PUB = sx.tile([32, FW], U32, tag="PUB")
            nc.vector.tensor_copy(PUB[:], P32)
            CMBB = sx.tile([32, FW, 8], F32, tag="CMBB")
            for f in range(FW):
                nc.gpsimd.indirect_dma_start(
                    out=CMBB[:, f, :], out_offset=None, in_=CMB[:],
                    in_offset=bass.IndirectOffsetOnAxis(ap=PUB[:, f:f + 1], axis=0))
            if past("s10c"):
                return
            if past("s10d"):
                return
            if past("s10e"):
                return
            SRCB = sx.tile([32, FW, 6], F32, tag="SRCB")
            nc.vector.tensor_copy(SRCB[:, :, 0:4], CMBB[:, :, 0:4])
            nc.vector.tensor_copy(SRCB[:, :, 4], K32)
            nc.vector.tensor_copy(SRCB[:, :, 5], CMBB[:, :, 4])
            if past("s10f"):
                return
            nc.sync.dma_start(outs["out"][:].rearrange("(p f) x -> p f x", f=FW),
                              SRCB[0:25, :, :])

        for _rep in range(reps):
            emit()


# ======================= host-side runner =======================
import concourse.tile as _tile
import concourse.bacc as _bacc

_CACHE = {}


def _build_nc(stop_after="full", reps=1):
    key = ("nc", stop_after, reps)
    if key not in _CACHE:
        nc = _bacc.Bacc("TRN2", target_bir_lowering=False, debug=False, num_devices=8)
        ins = {
            "lg": nc.dram_tensor("lg", [NT * P, NC], F32, kind="ExternalInput").ap(),
            "rl": nc.dram_tensor("rl", [NT * P, 4], F32, kind="ExternalInput").ap(),
            "an": nc.dram_tensor("an", [NT * P, 4], F32, kind="ExternalInput").ap(),
        }
        outs = {"out": nc.dram_tensor("out", [OUTN, 6], F32, kind="ExternalOutput").ap()}
        with _tile.TileContext(nc) as tc:
            build_kernel(tc, outs, ins, stop_after=stop_after, reps=reps)
        nc.compile()
        _CACHE[key] = nc
    return _CACHE[key]


def _make_runner(nc, n_cores=8):
    """Jitted SPMD executor for nc - traced/compiled ONCE, reused per call."""
    import jax
    from jax.experimental.shard_map import shard_map
    from jax.sharding import Mesh, PartitionSpec
    from concourse import bass2jax as b2j
    import concourse.mybir as _mybir

    b2j.install_neuronx_cc_hook()
    assert nc.dbg_addr is None and not nc.dbg_callbacks
    partition_name = nc.partition_id_tensor.name if nc.partition_id_tensor else None
    in_names, out_names, out_avals, out_shapes = [], [], [], []
    for alloc in nc.m.functions[0].allocations:
        if not isinstance(alloc, _mybir.MemoryLocationSet):
            continue
        name = alloc.memorylocations[0].name
        if alloc.kind == "ExternalInput":
            if name != partition_name:
                in_names.append(name)
        elif alloc.kind == "ExternalOutput":
            out_names.append(name)
            shape = tuple(alloc.tensor_shape)
            dtype = _mybir.dt.np(alloc.dtype)
            out_avals.append(jax.core.ShapedArray(shape, dtype))
            out_shapes.append((shape, dtype))
    n_params = len(in_names)
    n_outs = len(out_names)
    all_in = list(in_names) + list(out_names)
    if partition_name is not None:
        all_in.append(partition_name)
    donate = tuple(range(n_params, n_params + n_outs))

    def _body(*args):
        operands = list(args)
        if partition_name is not None:
            operands.append(b2j.partition_id_tensor())
        outs2 = b2j._bass_exec_p.bind(
            *operands, out_avals=tuple(out_avals), in_names=tuple(all_in),
            out_names=tuple(out_names), lowering_input_output_aliases=(),
            sim_require_finite=True, sim_require_nnan=True, nc=nc)
        return tuple(outs2)

    devices = jax.devices()[:n_cores]
    mesh = Mesh(np.asarray(devices), ("core",))
    in_specs = (PartitionSpec("core"),) * (n_params + n_outs)
    out_specs = (PartitionSpec("core"),) * n_outs
    sharded = jax.jit(
        shard_map(_body, mesh=mesh, in_specs=in_specs, out_specs=out_specs,
                  check_rep=False),
        donate_argnums=donate, keep_unused=True)

    def run(in_maps):
        per_core = [[np.asarray(m[name]) for name in in_names] for m in in_maps]
        concat_in = [np.concatenate([pc[i] for pc in per_core], axis=0)
                     for i in range(n_params)]
        concat_zeros = [np.zeros((n_cores * s[0], *s[1:]), d)
                        for s, d in out_shapes]
        out_arrs = sharded(*concat_in, *concat_zeros)
        return [{name: np.asarray(out_arrs[i]).reshape(n_cores, *out_shapes[i][0])[c]
                 for i, name in enumerate(out_names)}
                for c in range(n_cores)]

    run.sharded = sharded
    run.mesh = mesh
    run.in_names = in_names
    run.out_shapes = out_shapes
    run.n_cores = n_cores
    return run


def _get_runner(stop_after="full", reps=1):
    key = ("runner", stop_after, reps)
    if key not in _CACHE:
        _CACHE[key] = _make_runner(_build_nc(stop_after, reps))
    return _CACHE[key]


def _pad_image(logits, rel, anchors_pad):
    NPAD = NT * P
    L = np.zeros((NPAD, NC), np.float32); L[:8732] = logits
    R = np.zeros((NPAD, 4), np.float32); R[:8732] = rel
    return {"lg": L, "rl": R, "an": anchors_pad}


def _in_maps(bbox_regression, cls_logits, anchors):
    NPAD = NT * P
    A = np.tile(np.array([0, 0, 1, 1], np.float32), (NPAD, 1))
    A[:8732] = anchors
    B = cls_logits.shape[0]
    return [_pad_image(cls_logits[b], bbox_regression[b], A) for b in range(B)]


def _run(bbox_regression, cls_logits, anchors, stop_after="full", reps=1):
    res = _get_runner(stop_after, reps)(_in_maps(bbox_regression, cls_logits, anchors))
    out = np.stack([r["out"] for r in res]).astype(np.float32)
    return out, res


def kernel(bbox_regression, cls_logits, anchors):
    out, _ = _run(np.asarray(bbox_regression), np.asarray(cls_logits),
                  np.asarray(anchors))
    return out


# revision 4
# speedup vs baseline: 396.1007x; 1.0511x over previous
"""Optimized Bass/Tile kernel body for SSD postprocess (one image per core).

Sort keys are softmax scores (no log space). Gathers are class-major direct:
offsets live in [90,16] SBUF tiles, one indirect DMA per column, outputs land
class-major (no DRAM relayout roundtrips). Elementwise work is split between
DVE and GPSIMD(Pool); PSUM->SBUF copies ride Act/Pool; sort runs on 1024
entries with precomputed direction masks and in-place predicated updates.

Emission order (= queue order) is tuned for overlap: setup first (overlaps
input DMA), box decode is emitted after the value-gather issue so it fills
the DVE bubble during gather latency.
"""
import numpy as np
import concourse.bass as bass
import concourse.mybir as mybir
from concourse.masks import make_identity

P = 128
NT = 69            # anchor tiles (8832 = 69*128)
NC = 91
C1 = 90
NANC = NT * P      # 8832
NCH = NANC // 8    # 1104 chunks of 8 consecutive anchors per class
MCH = 12           # top chunks per class
MEL = 12           # top elements per class
MUSE = 11          # columns merged globally (90*11 = 990)
CAP = 1024
CLOG = 10          # log2(CAP)
FW = 8             # sort cols per partition (CAP = 128*FW)
OUTN = 200
NEG = -1e30
SCORE_THRESH = 0.01
BBOX_CLIP = float(np.log(1000.0 / 16.0))

F32 = mybir.dt.float32
U32 = mybir.dt.uint32
OP = mybir.AluOpType
AF = mybir.ActivationFunctionType
AX = mybir.AxisListType

STAGES = ["s1", "s2b", "s2", "s3", "s4", "s5", "s6", "s7", "s8", "s9", "s10a", "s10b", "s10c", "s10d", "s10e", "s10f", "full"]


def build_kernel(tc, outs, ins, dbg=None, stop_after="full", reps=1):
    nc = tc.nc
    dbg = dbg or {}
    LIMIT = STAGES.index(stop_after)

    def past(stage):
        return STAGES.index(stage) > LIMIT

    def dump(name, ap):
        if name in dbg:
            nc.sync.dma_start(dbg[name][:], ap)

    # DRAM scratch (offset-0 tensors; indirect DMA requires offset==0)
    TDC = nc.dram_tensor("tdc_scratch", [C1 * NCH, 8], F32, kind="Internal").ap()
    BD = nc.dram_tensor("bd_scratch", [NANC, 4], F32, kind="Internal").ap()
    CMB = nc.dram_tensor("cmb_scratch", [C1 * MUSE, 8], F32, kind="Internal").ap()
    SK = nc.dram_tensor("sk_scratch", [94 * MUSE], F32, kind="Internal").ap()
    SPd = nc.dram_tensor("sp_scratch", [94 * MUSE], F32, kind="Internal").ap()

    with tc.tile_pool(name="big", bufs=1) as bp, \
         tc.tile_pool(name="sm", bufs=2) as sm, \
         tc.tile_pool(name="sx", bufs=1) as sx, \
         tc.tile_pool(name="ps", bufs=2, space="PSUM") as ps, \
         tc.tile_pool(name="psb", bufs=4, space="PSUM") as psb:

        def emit():
            # ================= S0: input-independent setup =================
            ident = bp.tile([P, P], F32, tag="ident")
            make_identity(nc, ident[:])
            IOTE = bp.tile([P, FW], U32, tag="IOTE")
            nc.gpsimd.iota(IOTE[:], pattern=[[1, FW]], base=0, channel_multiplier=FW)
            BITS = []
            for b in range(CLOG + 1):
                bt = bp.tile([P, FW], U32, tag=f"BITS{b}")
                if b == CLOG:
                    nc.vector.memset(bt[:], 0)
                else:
                    nc.vector.tensor_scalar(bt[:], IOTE[:], b, 1,
                                            op0=OP.logical_shift_right,
                                            op1=OP.bitwise_and)
                BITS.append(bt)
            WMIN = {}
            for klog in range(1, CLOG + 1):
                for jlog in range(klog - 1, -1, -1):
                    wt = bp.tile([P, FW], U32, tag=f"WM{klog}_{jlog}")
                    nc.vector.tensor_tensor(wt[:], BITS[klog][:], BITS[jlog][:],
                                            op=OP.logical_xor)
                    WMIN[(klog, jlog)] = wt
            CTu = bp.tile([P, P], U32, tag="CTu")
            nc.gpsimd.iota(CTu[:], pattern=[[1, P]], base=0, channel_multiplier=0)
            RTu = bp.tile([P, P], U32, tag="RTu")
            nc.gpsimd.iota(RTu[:], pattern=[[0, P]], base=0, channel_multiplier=1)
            nc.vector.tensor_tensor(CTu[:], CTu[:], RTu[:], op=OP.bitwise_xor)
            PERMS = []
            for b in range(7):
                pm = bp.tile([P, P], F32, tag=f"PERM{b}")
                pu = sm.tile([P, P], U32, tag="pu")
                nc.vector.tensor_scalar(pu[:], CTu[:], 1 << b, None, op0=OP.is_equal)
                nc.vector.tensor_copy(pm[:], pu[:])
                PERMS.append(pm)
            # shift matrices for the S10 cross-partition tie repair
            CT2 = bp.tile([P, P], U32, tag="CT2")
            nc.gpsimd.iota(CT2[:], pattern=[[1, P]], base=0, channel_multiplier=0)
            shm = sm.tile([P, P], U32, tag="shm")
            nc.vector.tensor_scalar(shm[:], CT2[:], 1, None, op0=OP.add)
            shu = sm.tile([P, P], U32, tag="shu")
            nc.vector.tensor_tensor(shu[:], RTu[:], shm[:], op=OP.is_equal)
            SU = bp.tile([P, P], F32, tag="SU")
            nc.vector.tensor_copy(SU[:], shu[:])
            nc.vector.tensor_scalar(shm[:], RTu[:], 1, None, op0=OP.add)
            nc.vector.tensor_tensor(shu[:], shm[:], CT2[:], op=OP.is_equal)
            SD = bp.tile([P, P], F32, tag="SD")
            nc.vector.tensor_copy(SD[:], shu[:])
            CBASE = bp.tile([C1, 1], U32, tag="CBASE")
            nc.gpsimd.iota(CBASE[:], pattern=[[0, 1]], base=0, channel_multiplier=NCH)
            KI = bp.tile([C1, MCH], U32, tag="KI")
            nc.gpsimd.iota(KI[:], pattern=[[1, MCH]], base=0, channel_multiplier=0)
            KIF = bp.tile([C1, MCH], F32, tag="KIF")
            nc.vector.tensor_copy(KIF[:], KI[:])
            E0 = bp.tile([C1, MUSE], U32, tag="E0")
            nc.gpsimd.iota(E0[:], pattern=[[1, MUSE]], base=0, channel_multiplier=MUSE)
            E0F = bp.tile([C1, MUSE], F32, tag="E0F")
            nc.vector.tensor_copy(E0F[:], E0[:])
            CLS1 = bp.tile([C1, 1], U32, tag="CLS1")
            nc.gpsimd.iota(CLS1[:], pattern=[[0, 1]], base=1, channel_multiplier=1)
            TRI = bp.tile([C1, MEL, MEL], F32, tag="TRI")
            nc.vector.memset(TRI[:].rearrange("c a b -> c (a b)"), 1.0)
            nc.gpsimd.affine_select(out=TRI[:], in_=TRI[:],
                                    pattern=[[-1, MEL], [1, MEL]],
                                    compare_op=OP.is_ge, fill=0.0, base=-1,
                                    channel_multiplier=0)
            MSKX = bp.tile([94, MUSE], F32, tag="MSKX")
            nc.vector.memset(MSKX[:], NEG)
            E0X = bp.tile([94, MUSE], F32, tag="E0X")
            nc.vector.memset(E0X[:], 0.0)
            nc.vector.tensor_copy(E0X[0:C1, :], E0F[:])

            # ====== S1+S2b: 3-chunk softmax -> transpose -> evac pipeline =====
            # Act's LG slice is issued first on its queue so exp starts the
            # moment it lands; payload staging rides sync instead.
            LG = bp.tile([P, NT, NC], F32, tag="LG")
            lgsrc = ins["lg"][:].rearrange("(t p) c -> p t c", p=P)
            CHS = [(0, 24), (24, 48), (48, NT)]
            nc.scalar.dma_start(LG[:, 0:24, :], lgsrc[:, 0:24, :])
            nc.sync.dma_start(LG[:, 24:48, :], lgsrc[:, 24:48, :])
            nc.gpsimd.dma_start(LG[:, 48:NT, :], lgsrc[:, 48:NT, :])
            RL = bp.tile([P, NT, 4], F32, tag="RL")
            nc.sync.dma_start(RL[:], ins["rl"][:].rearrange("(t p) c -> p t c", p=P))
            AN = bp.tile([P, NT, 4], F32, tag="AN")
            nc.gpsimd.dma_start(AN[:], ins["an"][:].rearrange("(t p) c -> p t c", p=P))
            nc.sync.dma_start(SPd.rearrange("(c m) -> c m", m=MUSE), E0X[:])

            E = bp.tile([P, NT, NC], F32, tag="E")
            ZT1 = bp.tile([P, NT, 46], F32, tag="ZT1")
            ZT2 = bp.tile([P, NT, 23], F32, tag="ZT2")
            Z = sm.tile([P, NT], F32, tag="Z")
            RZ = sm.tile([P, NT], F32, tag="RZ")
            TCM = bp.tile([C1, NANC], F32, tag="TCM")  # classes 1..90 on parts 0..89
            tdc = TDC.rearrange("(c h) i -> c (h i)", c=C1)
            gidx = 0
            for ci, (ta, tb) in enumerate(CHS):
                w = tb - ta
                nc.scalar.activation(E[:, ta:tb, :], LG[:, ta:tb, :], AF.Exp)
                eng = nc.vector if ci % 2 == 0 else nc.gpsimd
                eng.tensor_tensor(ZT1[:, ta:tb, 0:45], E[:, ta:tb, 0:45],
                                  E[:, ta:tb, 46:91], op=OP.add)
                eng.tensor_copy(ZT1[:, ta:tb, 45], E[:, ta:tb, 45])
                eng.tensor_tensor(ZT2[:, ta:tb, :], ZT1[:, ta:tb, 0:23],
                                  ZT1[:, ta:tb, 23:46], op=OP.add)
                if ci % 2 == 0:
                    nc.vector.tensor_reduce(Z[:, ta:tb], ZT2[:, ta:tb, :],
                                            axis=AX.X, op=OP.add)
                    nc.vector.reciprocal(RZ[:, ta:tb], Z[:, ta:tb])
                    nc.vector.tensor_tensor(
                        E[:, ta:tb, :], E[:, ta:tb, :],
                        RZ[:, ta:tb].to_broadcast([P, w, NC]), op=OP.mult)
                else:
                    # Pool lacks free-axis reduce: finish the tree pairwise
                    ZT3 = sm.tile([P, NT, 12], F32, tag="ZT3")
                    nc.gpsimd.tensor_tensor(ZT3[:, ta:tb, 0:11], ZT2[:, ta:tb, 0:11],
                                            ZT2[:, ta:tb, 12:23], op=OP.add)
                    nc.gpsimd.tensor_copy(ZT3[:, ta:tb, 11], ZT2[:, ta:tb, 11])
                    ZT4 = sm.tile([P, NT, 6], F32, tag="ZT4")
                    nc.gpsimd.tensor_tensor(ZT4[:, ta:tb, :], ZT3[:, ta:tb, 0:6],
                                            ZT3[:, ta:tb, 6:12], op=OP.add)
                    ZT5 = sm.tile([P, NT, 3], F32, tag="ZT5")
                    nc.gpsimd.tensor_tensor(ZT5[:, ta:tb, :], ZT4[:, ta:tb, 0:3],
                                            ZT4[:, ta:tb, 3:6], op=OP.add)
                    nc.gpsimd.tensor_tensor(Z[:, ta:tb], ZT5[:, ta:tb, 0],
                                            ZT5[:, ta:tb, 1], op=OP.add)
                    nc.gpsimd.tensor_tensor(Z[:, ta:tb], Z[:, ta:tb],
                                            ZT5[:, ta:tb, 2], op=OP.add)
                    nc.vector.reciprocal(RZ[:, ta:tb], Z[:, ta:tb])
                    nc.gpsimd.tensor_tensor(
                        E[:, ta:tb, :], E[:, ta:tb, :],
                        RZ[:, ta:tb].to_broadcast([P, w, NC]), op=OP.mult)
                for t0 in range(ta, tb, 4):
                    n = min(4, tb - t0)
                    pt = ps.tile([C1, 4, P], F32, tag="trp", space="PSUM")
                    for j in range(n):
                        nc.tensor.transpose(out=pt[:, j, :], in_=E[:, t0 + j, 1:NC],
                                            identity=ident[:])
                    if gidx % 2 == 0:
                        nc.scalar.copy(TCM[:, t0 * P:(t0 + n) * P],
                                       pt[:, 0:n, :].rearrange("c a b -> c (a b)"))
                    else:
                        nc.vector.tensor_copy(
                            TCM[:, t0 * P:(t0 + n) * P],
                            pt[:, 0:n, :].rearrange("c a b -> c (a b)"))
                    gidx += 1
                q = [nc.sync, nc.gpsimd, nc.scalar][ci]
                q.dma_start(tdc[:, ta * P:tb * P], TCM[:, ta * P:tb * P])
            dump("T", E[:])
            dump("TCM", TCM[:])

            if past("s1"):
                return
            if past("s2b"):
                return
            # ===== S2: decode boxes (emitted here to fill the gather bubble) ==
            def col(t, k):
                return t[:, :, k]

            BX = bp.tile([P, NT, 4], F32, tag="BX")
            aw = sm.tile([P, NT], F32, tag="aw")
            nc.vector.tensor_tensor(aw[:], col(AN, 2), col(AN, 0), op=OP.subtract)
            ah = sm.tile([P, NT], F32, tag="ah")
            nc.gpsimd.tensor_tensor(ah[:], col(AN, 3), col(AN, 1), op=OP.subtract)
            ax = sm.tile([P, NT], F32, tag="ax")
            nc.vector.tensor_scalar(ax[:], aw[:], 0.5, None, op0=OP.mult)
            nc.vector.tensor_tensor(ax[:], ax[:], col(AN, 0), op=OP.add)
            ay = sm.tile([P, NT], F32, tag="ay")
            nc.gpsimd.tensor_scalar(ay[:], ah[:], 0.5, None, op0=OP.mult)
            nc.gpsimd.tensor_tensor(ay[:], ay[:], col(AN, 1), op=OP.add)

            dx = sm.tile([P, NT], F32, tag="dx")
            nc.vector.tensor_scalar(dx[:], col(RL, 0), 0.1, None, op0=OP.mult)
            px = sm.tile([P, NT], F32, tag="px")
            nc.vector.tensor_tensor(px[:], dx[:], aw[:], op=OP.mult)
            nc.vector.tensor_tensor(px[:], px[:], ax[:], op=OP.add)
            dw = sm.tile([P, NT], F32, tag="dw")
            nc.vector.tensor_scalar(dw[:], col(RL, 2), 0.2, BBOX_CLIP, op0=OP.mult,
                                    op1=OP.min)
            ew = sm.tile([P, NT], F32, tag="ew")
            nc.scalar.activation(ew[:], dw[:], AF.Exp)
            pw = sm.tile([P, NT], F32, tag="pw")
            nc.vector.tensor_tensor(pw[:], ew[:], aw[:], op=OP.mult)
            nc.vector.tensor_scalar(pw[:], pw[:], 0.5, None, op0=OP.mult)

            dy = sm.tile([P, NT], F32, tag="dy")
            nc.gpsimd.tensor_scalar(dy[:], col(RL, 1), 0.1, None, op0=OP.mult)
            py = sm.tile([P, NT], F32, tag="py")
            nc.gpsimd.tensor_tensor(py[:], dy[:], ah[:], op=OP.mult)
            nc.gpsimd.tensor_tensor(py[:], py[:], ay[:], op=OP.add)
            dh = sm.tile([P, NT], F32, tag="dh")
            nc.gpsimd.tensor_scalar(dh[:], col(RL, 3), 0.2, BBOX_CLIP, op0=OP.mult,
                                    op1=OP.min)
            eh = sm.tile([P, NT], F32, tag="eh")
            nc.scalar.activation(eh[:], dh[:], AF.Exp)
            ph = sm.tile([P, NT], F32, tag="ph")
            nc.gpsimd.tensor_tensor(ph[:], eh[:], ah[:], op=OP.mult)
            nc.gpsimd.tensor_scalar(ph[:], ph[:], 0.5, None, op0=OP.mult)

            tq = sm.tile([P, NT], F32, tag="tq")
            nc.vector.tensor_tensor(tq[:], px[:], pw[:], op=OP.subtract)
            nc.vector.tensor_scalar(col(BX, 0), tq[:], 0.0, 300.0, op0=OP.max,
                                    op1=OP.min)
            nc.vector.tensor_tensor(tq[:], px[:], pw[:], op=OP.add)
            nc.vector.tensor_scalar(col(BX, 2), tq[:], 0.0, 300.0, op0=OP.max,
                                    op1=OP.min)
            tq2 = sm.tile([P, NT], F32, tag="tq2")
            nc.gpsimd.tensor_tensor(tq2[:], py[:], ph[:], op=OP.subtract)
            nc.gpsimd.tensor_scalar(col(BX, 1), tq2[:], 0.0, 300.0, op0=OP.max,
                                    op1=OP.min)
            nc.gpsimd.tensor_tensor(tq2[:], py[:], ph[:], op=OP.add)
            nc.gpsimd.tensor_scalar(col(BX, 3), tq2[:], 0.0, 300.0, op0=OP.max,
                                    op1=OP.min)
            nc.sync.dma_start(BD.rearrange("(t p) c -> p t c", p=P), BX[:])
            dump("BX", BX[:])

            if past("s2"):
                return
            # ====== S3: chunk max via pairwise-max tree (DVE; Pool lacks
            # tensor_tensor max) — 3 halving levels beat one 8-wide reduce ====
            t4 = TCM[:].rearrange("c (h two) -> c h two", two=2)   # [C1, 4416, 2]
            L1 = bp.tile([C1, NCH * 4], F32, tag="L1")
            nc.vector.tensor_tensor(L1[:], t4[:, :, 0], t4[:, :, 1], op=OP.max)
            l4 = L1[:].rearrange("c (h two) -> c h two", two=2)    # [C1, 2208, 2]
            L2 = bp.tile([C1, NCH * 2], F32, tag="L2")
            nc.vector.tensor_tensor(L2[:], l4[:, :, 0], l4[:, :, 1], op=OP.max)
            l5 = L2[:].rearrange("c (h two) -> c h two", two=2)    # [C1, 1104, 2]
            CC = bp.tile([C1, NCH], F32, tag="CC")
            nc.vector.tensor_tensor(CC[:], l5[:, :, 0], l5[:, :, 1], op=OP.max)
            dump("CC", CC[:])

            if past("s3"):
                return
            # ================= S4: top-16 chunks per class ====================
            CHI = bp.tile([C1, MCH], U32, tag="CHI")
            CHIF = bp.tile([C1, MCH], F32, tag="CHIF")
            OFF1 = bp.tile([C1, MCH], U32, tag="OFF1")
            GV = bp.tile([C1, MCH, 8], F32, tag="GV")
            for r in range(2):
                lo, hi = 8 * r, min(8 * r + 8, MCH)
                m8 = sm.tile([C1, 8], F32, tag="m8")
                nc.vector.max(out=m8[:], in_=CC[:])
                i8 = sm.tile([C1, 8], U32, tag="i8")
                nc.vector.max_index(out=i8[:], in_max=m8[:], in_values=CC[:])
                nc.vector.tensor_copy(CHI[:, lo:hi], i8[:, 0:hi - lo])
                if r < 1:
                    nc.vector.match_replace(out=CC[:], in_to_replace=m8[:],
                                            in_values=CC[:], imm_value=NEG)
                # launch this round's gathers immediately (overlaps next round)
                nc.vector.tensor_tensor(OFF1[:, lo:hi], CHI[:, lo:hi],
                                        CBASE[:].to_broadcast([C1, hi - lo]),
                                        op=OP.add)
                for t in range(lo, hi):
                    nc.gpsimd.indirect_dma_start(
                        out=GV[:, t, :], out_offset=None, in_=TDC[:],
                        in_offset=bass.IndirectOffsetOnAxis(ap=OFF1[:, t:t + 1],
                                                            axis=0))
            nc.vector.tensor_copy(CHIF[:], CHI[:])
            dump("CHI", CHI[:])

            if past("s4"):
                return
            # ========== S5: gathers launched above (class-major direct) =======
            dump("GV", GV[:].rearrange("c k i -> c (k i)"))

            if past("s5"):
                return
            # ========= S6: top-16 elements; anchors via index formula =========
            GVf = GV[:].rearrange("c k i -> c (k i)")
            GVK = bp.tile([C1, MCH * 8], F32, tag="GVK")
            nc.vector.tensor_copy(GVK[:], GVf)
            VAL = bp.tile([C1, MEL], F32, tag="VAL")
            IDX = bp.tile([C1, MEL], U32, tag="IDX")
            for r in range(2):
                m8b = sm.tile([C1, 8], F32, tag="m8b")
                nc.vector.max(out=m8b[:], in_=GVK[:])
                i8b = sm.tile([C1, 8], U32, tag="i8b")
                nc.vector.max_index(out=i8b[:], in_max=m8b[:], in_values=GVK[:])
                w = min(8, MEL - 8 * r)
                nc.vector.tensor_copy(VAL[:, 8 * r:8 * r + w], m8b[:, 0:w])
                nc.vector.tensor_copy(IDX[:, 8 * r:8 * r + w], i8b[:, 0:w])
                if r < 1:
                    nc.vector.match_replace(out=GVK[:], in_to_replace=m8b[:],
                                            in_values=GVK[:], imm_value=NEG)
            dump("VAL", VAL[:])
            KHI = sm.tile([C1, MEL], U32, tag="KHI")
            nc.vector.tensor_scalar(KHI[:], IDX[:], 3, None, op0=OP.logical_shift_right)
            KLO = sm.tile([C1, MEL], U32, tag="KLO")
            nc.vector.tensor_scalar(KLO[:], IDX[:], 7, None, op0=OP.bitwise_and)
            KLOF = sm.tile([C1, MEL], F32, tag="KLOF")
            nc.vector.tensor_copy(KLOF[:], KLO[:])
            KHIF = sm.tile([C1, MEL], F32, tag="KHIF")
            nc.vector.tensor_copy(KHIF[:], KHI[:])
            EQ = sx.tile([C1, MEL, MCH], F32, tag="EQ")
            nc.vector.tensor_tensor(EQ[:], KHIF[:].to_broadcast([C1, MEL, MCH]),
                                    KIF[:].to_broadcast([C1, MCH, MEL]).rearrange(
                                        "c a b -> c b a"), op=OP.is_equal)
            nc.vector.tensor_tensor(EQ[:], EQ[:],
                                    CHIF[:].to_broadcast([C1, MCH, MEL]).rearrange(
                                        "c a b -> c b a"), op=OP.mult)
            ANCF = bp.tile([C1, MEL], F32, tag="ANCF")
            nc.vector.tensor_reduce(ANCF[:], EQ[:], axis=AX.X, op=OP.add)
            nc.vector.tensor_scalar(ANCF[:], ANCF[:], 8.0, None, op0=OP.mult)
            nc.vector.tensor_tensor(ANCF[:], ANCF[:], KLOF[:], op=OP.add)
            ANC = bp.tile([C1, MEL], U32, tag="ANC")
            nc.vector.tensor_copy(ANC[:], ANCF[:])
            dump("ANC", ANC[:])
            dump("IDX", IDX[:])
            dump("ANCF", ANCF[:])

            if past("s6"):
                return
            # ========= S7: gather boxes (class-major direct), greedy NMS ======
            BOXG = bp.tile([C1, MEL, 4], F32, tag="BOXG")
            for t in range(MEL):
                nc.gpsimd.indirect_dma_start(
                    out=BOXG[:, t, :], out_offset=None, in_=BD[:],
                    in_offset=bass.IndirectOffsetOnAxis(ap=ANC[:, t:t + 1], axis=0))
            x1 = BOXG[:, :, 0]; y1 = BOXG[:, :, 1]; x2 = BOXG[:, :, 2]; y2 = BOXG[:, :, 3]
            AREA = sm.tile([C1, MEL], F32, tag="AREA")
            wq = sm.tile([C1, MEL], F32, tag="wq")
            nc.vector.tensor_tensor(wq[:], x2, x1, op=OP.subtract)
            nc.vector.tensor_tensor(AREA[:], y2, y1, op=OP.subtract)
            nc.vector.tensor_tensor(AREA[:], AREA[:], wq[:], op=OP.mult)

            def bi(apv):
                return apv.to_broadcast([C1, MEL, MEL])

            def bj(apv):
                return apv.to_broadcast([C1, MEL, MEL]).rearrange("c a b -> c b a")

            AM = bp.tile([C1, MEL, MEL], F32, tag="AM")
            W1 = bp.tile([C1, MEL, MEL], F32, tag="W1")
            W2 = bp.tile([C1, MEL, MEL], F32, tag="W2")
            nc.vector.tensor_tensor(W1[:], bi(x1), bj(x1), op=OP.max)
            nc.vector.tensor_tensor(W2[:], bi(x2), bj(x2), op=OP.min)
            nc.vector.tensor_tensor(W1[:], W2[:], W1[:], op=OP.subtract)
            nc.vector.tensor_scalar(W1[:], W1[:], 0.0, None, op0=OP.max)
            nc.vector.tensor_tensor(AM[:], bi(y1), bj(y1), op=OP.max)
            nc.vector.tensor_tensor(W2[:], bi(y2), bj(y2), op=OP.min)
            nc.vector.tensor_tensor(AM[:], W2[:], AM[:], op=OP.subtract)
            nc.vector.tensor_scalar(AM[:], AM[:], 0.0, None, op0=OP.max)
            nc.vector.tensor_tensor(W1[:], W1[:], AM[:], op=OP.mult)
            nc.vector.tensor_tensor(W2[:], bi(AREA[:]), bj(AREA[:]), op=OP.add)
            nc.vector.tensor_tensor(W2[:], W2[:], W1[:], op=OP.subtract)
            nc.vector.tensor_scalar(W2[:], W2[:], 0.45, 0.45e-8, op0=OP.mult,
                                    op1=OP.add)
            nc.vector.tensor_tensor(AM[:], W1[:], W2[:], op=OP.is_gt)
            nc.vector.tensor_tensor(AM[:], AM[:], TRI[:], op=OP.mult)
            dump("AM", AM[:])

            KEEP = bp.tile([C1, MEL], F32, tag="KEEP")
            nc.vector.memset(KEEP[:], 1.0)
            tk = sm.tile([C1, MEL], F32, tag="tk")
            for i in range(MEL - 1):
                nc.vector.scalar_tensor_tensor(out=tk[:], in0=AM[:, i, :],
                                               scalar=KEEP[:, i:i + 1], in1=KEEP[:],
                                               op0=OP.mult, op1=OP.mult)
                nc.vector.tensor_tensor(KEEP[:], KEEP[:], tk[:], op=OP.subtract)
            dump("KEEP", KEEP[:])

            if past("s7"):
                return
            # ================= S8: mask, stage merge arrays ===================
            CNDu = sm.tile([C1, MEL], U32, tag="CNDu")
            nc.vector.tensor_scalar(CNDu[:], VAL[:], SCORE_THRESH, None, op0=OP.is_gt)
            KEEPu = sm.tile([C1, MEL], U32, tag="KEEPu")
            nc.vector.tensor_copy(KEEPu[:], KEEP[:])
            nc.vector.tensor_tensor(CNDu[:], CNDu[:], KEEPu[:], op=OP.logical_and)
            MSK = bp.tile([C1, MEL], F32, tag="MSK")
            nc.vector.memset(MSK[:], NEG)
            nc.vector.copy_predicated(MSK[:], CNDu[:], VAL[:])
            dump("MSK", MSK[:])

            # combined per-candidate record: box(4) + label(1); one gather in S10
            CMBT = bp.tile([C1, MUSE, 8], F32, tag="CMBT")
            nc.gpsimd.memset(CMBT[:].rearrange("c m x -> c (m x)"), 0.0)
            nc.gpsimd.tensor_copy(CMBT[:, :, 0:4], BOXG[:, 0:MUSE, :])
            CLS1F = sm.tile([C1, 1], F32, tag="CLS1F")
            nc.gpsimd.tensor_copy(CLS1F[:], CLS1[:])
            nc.gpsimd.tensor_copy(CMBT[:, :, 4], CLS1F[:].to_broadcast([C1, MUSE]))
            nc.gpsimd.dma_start(CMB.rearrange("(c m) x -> c m x", m=MUSE), CMBT[:])
            nc.vector.tensor_copy(MSKX[0:C1, :], MSK[:, :MUSE])
            nc.sync.dma_start(SK.rearrange("(c m) -> c m", m=MUSE), MSKX[:])

            if past("s8"):
                return
            # ===== S9: bitonic sort 1024 desc; keys/payload in separate ====
            # tiles so stage t+1's key permute (PE) overlaps stage t's payload
            # update (DVE)
            KPK = bp.tile([P, FW], F32, tag="KPK")
            KPP = bp.tile([P, FW], F32, tag="KPP")
            nc.sync.dma_start(KPK[:], SK[0:CAP].rearrange("(p f) -> p f", f=FW))
            nc.scalar.dma_start(KPP[:], SPd[0:CAP].rearrange("(p f) -> p f", f=FW))
            for klog in range(1, CLOG + 1):
                for jlog in range(klog - 1, -1, -1):
                    wmin = WMIN[(klog, jlog)]
                    pmk = psb.tile([P, FW], F32, tag="pmk", space="PSUM")
                    pmp = psb.tile([P, FW], F32, tag="pmp", space="PSUM")
                    if jlog < 3:
                        j = 1 << jlog
                        kv = KPK[:].rearrange("p (a s l) -> p a s l", s=2, l=j)
                        pv = KPP[:].rearrange("p (a s l) -> p a s l", s=2, l=j)
                        nc.tensor.matmul(
                            pmk[:].rearrange("p (a s l) -> p a s l", s=2, l=j),
                            lhsT=ident[:], rhs=kv[:, :, ::-1, :],
                            start=True, stop=True)
                        nc.tensor.matmul(
                            pmp[:].rearrange("p (a s l) -> p a s l", s=2, l=j),
                            lhsT=ident[:], rhs=pv[:, :, ::-1, :],
                            start=True, stop=True)
                    else:
                        nc.tensor.matmul(pmk[:], lhsT=PERMS[jlog - 3][:], rhs=KPK[:],
                                         start=True, stop=True)
                        nc.tensor.matmul(pmp[:], lhsT=PERMS[jlog - 3][:], rhs=KPP[:],
                                         start=True, stop=True)
                    sw = sm.tile([P, FW], U32, tag="sw")
                    nc.vector.tensor_tensor(sw[:], pmk[:], KPK[:], op=OP.is_gt)
                    nc.vector.tensor_tensor(sw[:], sw[:], wmin[:], op=OP.logical_xor)
                    nc.vector.copy_predicated(KPK[:], sw[:], pmk[:])
                    nc.vector.copy_predicated(KPP[:], sw[:], pmp[:])
            dump("SORTEDK", KPK[:])
            dump("SORTEDP", KPP[:])

            if past("s9"):
                return
            # ========== S10: tie repair in [32,8], direct output ==============
            K32 = KPK[0:32, :]
            P32 = KPP[0:32, :]
            kvA = K32.rearrange("p (q two) -> p q two", two=2)
            pvA = P32.rearrange("p (q two) -> p q two", two=2)
            eqA = sx.tile([32, 4], U32, tag="eqA")
            nc.vector.tensor_tensor(eqA[:], kvA[:, :, 0], kvA[:, :, 1], op=OP.is_equal)
            gtA = sx.tile([32, 4], U32, tag="gtA")
            nc.vector.tensor_tensor(gtA[:], pvA[:, :, 0], pvA[:, :, 1], op=OP.is_gt)
            nc.vector.tensor_tensor(eqA[:], eqA[:], gtA[:], op=OP.logical_and)
            swpA = sx.tile([32, 4, 2], F32, tag="swpA")
            nc.vector.tensor_copy(swpA[:, :, 0], pvA[:, :, 1])
            nc.vector.tensor_copy(swpA[:, :, 1], pvA[:, :, 0])
            cndA = sx.tile([32, 4, 2], U32, tag="cndA")
            nc.vector.tensor_copy(cndA[:, :, 0], eqA[:])
            nc.vector.tensor_copy(cndA[:, :, 1], eqA[:])
            nc.vector.copy_predicated(pvA[:], cndA[:], swpA[:])
            kvB = K32[:, 1:7].rearrange("p (q two) -> p q two", two=2)
            pvB = P32[:, 1:7].rearrange("p (q two) -> p q two", two=2)
            eqB = sx.tile([32, 3], U32, tag="eqB")
            nc.vector.tensor_tensor(eqB[:], kvB[:, :, 0], kvB[:, :, 1], op=OP.is_equal)
            gtB = sx.tile([32, 3], U32, tag="gtB")
            nc.vector.tensor_tensor(gtB[:], pvB[:, :, 0], pvB[:, :, 1], op=OP.is_gt)
            nc.vector.tensor_tensor(eqB[:], eqB[:], gtB[:], op=OP.logical_and)
            swpB = sx.tile([32, 3, 2], F32, tag="swpB")
            nc.vector.tensor_copy(swpB[:, :, 0], pvB[:, :, 1])
            nc.vector.tensor_copy(swpB[:, :, 1], pvB[:, :, 0])
            cndB = sx.tile([32, 3, 2], U32, tag="cndB")
            nc.vector.tensor_copy(cndB[:, :, 0], eqB[:])
            nc.vector.tensor_copy(cndB[:, :, 1], eqB[:])
            nc.vector.copy_predicated(pvB[:], cndB[:], swpB[:])
            if past("s10a"):
                return
            # parity B boundary: (p,7) vs (p+1,0) via PE shift (no DRAM trip)
            pshU = psb.tile([P, FW], F32, tag="pmk", space="PSUM")
            nc.tensor.matmul(pshU[:, 0:1], lhsT=SU[:], rhs=KPK[:, 0:1],
                             start=True, stop=True)
            nc.tensor.matmul(pshU[:, 1:2], lhsT=SU[:], rhs=KPP[:, 0:1],
                             start=True, stop=True)
            eqC = sx.tile([32, 1], U32, tag="eqC")
            nc.vector.tensor_tensor(eqC[:], K32[:, 7:8], pshU[0:32, 0:1],
                                    op=OP.is_equal)
            gtC = sx.tile([32, 1], U32, tag="gtC")
            nc.vector.tensor_tensor(gtC[:], P32[:, 7:8], pshU[0:32, 1:2],
                                    op=OP.is_gt)
            nc.vector.tensor_tensor(eqC[:], eqC[:], gtC[:], op=OP.logical_and)
            PB7 = sx.tile([P, 2], F32, tag="PB7")
            nc.vector.memset(PB7[:], 0.0)
            nc.vector.tensor_copy(PB7[0:32, 0:1], P32[:, 7:8])
            nc.vector.tensor_copy(PB7[0:32, 1:2], eqC[:])
            nc.vector.copy_predicated(P32[:, 7:8], eqC[:], pshU[0:32, 1:2])
            pshD = psb.tile([P, FW], F32, tag="pmp", space="PSUM")
            nc.tensor.matmul(pshD[:, 0:2], lhsT=SD[:], rhs=PB7[:], start=True,
                             stop=True)
            eq2 = sx.tile([32, 1], U32, tag="eq2")
            nc.vector.tensor_copy(eq2[:], pshD[0:32, 1:2])
            nc.vector.copy_predicated(P32[:, 0:1], eq2[:], pshD[0:32, 0:1])
            dump("KR", K32)
            dump("PR", P32)
            if past("s10b"):
                return

            PUB = sx.tile([32, FW], U32, tag="PUB")
            nc.vector.tensor_copy(PUB[:], P32)
            CMBB = sx.tile([32, FW, 8], F32, tag="CMBB")
            for f in range(FW):
                nc.gpsimd.indirect_dma_start(
                    out=CMBB[:, f, :], out_offset=None, in_=CMB[:],
                    in_offset=bass.IndirectOffsetOnAxis(ap=PUB[:, f:f + 1], axis=0))
            if past("s10c"):
                return
            if past("s10d"):
                return
            if past("s10e"):
                return
            SRCB = sx.tile([32, FW, 6], F32, tag="SRCB")
            nc.vector.tensor_copy(SRCB[:, :, 0:4], CMBB[:, :, 0:4])
            nc.vector.tensor_copy(SRCB[:, :, 4], K32)
            nc.vector.tensor_copy(SRCB[:, :, 5], CMBB[:, :, 4])
            if past("s10f"):
                return
            nc.sync.dma_start(outs["out"][:].rearrange("(p f) x -> p f x", f=FW),
                              SRCB[0:25, :, :])

        for _rep in range(reps):
            emit()


# ======================= host-side runner =======================
import concourse.tile as _tile
import concourse.bacc as _bacc

_CACHE = {}


def _build_nc(stop_after="full", reps=1):
    key = ("nc", stop_after, reps)
    if key not in _CACHE:
        nc = _bacc.Bacc("TRN2", target_bir_lowering=False, debug=False, num_devices=8)
        ins = {
            "lg": nc.dram_tensor("lg", [NT * P, NC], F32, kind="ExternalInput").ap(),
            "rl": nc.dram_tensor("rl", [NT * P, 4], F32, kind="ExternalInput").ap(),
            "an": nc.dram_tensor("an", [NT * P, 4], F32, kind="ExternalInput").ap(),
        }
        outs = {"out": nc.dram_tensor("out", [OUTN, 6], F32, kind="ExternalOutput").ap()}
        with _tile.TileContext(nc) as tc:
            build_kernel(tc, outs, ins, stop_after=stop_after, reps=reps)
        nc.compile()
        _CACHE[key] = nc
    return _CACHE[key]


def _make_runner(nc, n_cores=8):
    """Jitted SPMD executor for nc - traced/compiled ONCE, reused per call."""
    import jax
    from jax.experimental.shard_map import shard_map
    from jax.sharding import Mesh, PartitionSpec
    from concourse import bass2jax as b2j
    import concourse.mybir as _mybir

    b2j.install_neuronx_cc_hook()
    assert nc.dbg_addr is None and not nc.dbg_callbacks
    partition_name = nc.partition_id_tensor.name if nc.partition_id_tensor else None
    in_names, out_names, out_avals, out_shapes = [], [], [], []
    for alloc in nc.m.functions[0].allocations:
        if not isinstance(alloc, _mybir.MemoryLocationSet):
            continue
        name = alloc.memorylocations[0].name
        if alloc.kind == "ExternalInput":
            if name != partition_name:
                in_names.append(name)
        elif alloc.kind == "ExternalOutput":
            out_names.append(name)
            shape = tuple(alloc.tensor_shape)
            dtype = _mybir.dt.np(alloc.dtype)
            out_avals.append(jax.core.ShapedArray(shape, dtype))
            out_shapes.append((shape, dtype))
    n_params = len(in_names)
    n_outs = len(out_names)
    all_in = list(in_names) + list(out_names)
    if partition_name is not None:
        all_in.append(partition_name)
    donate = tuple(range(n_params, n_params + n_outs))

    def _body(*args):
        operands = list(args)
        if partition_name is not None:
            operands.append(b2j.partition_id_tensor())
        outs2 = b2j._bass_exec_p.bind(
            *operands, out_avals=tuple(out_avals), in_names=tuple(all_in),
            out_names=tuple(out_names), lowering_input_output_aliases=(),
            sim_require_finite=True, sim_require_nnan=True, nc=nc)
        return tuple(outs2)

    devices = jax.devices()[:n_cores]
    mesh = Mesh(np.asarray(devices), ("core",))
    in_specs = (PartitionSpec("core"),) * (n_params + n_outs)
    out_specs = (PartitionSpec("core"),) * n_outs
    sharded = jax.jit(
        shard_map(_body, mesh=mesh, in_specs=in_specs, out_specs=out_specs,
                  check_rep=False),
        donate_argnums=donate, keep_unused=True)

    def run(in_maps):
        per_core = [[np.asarray(m[name]) for name in in_names] for m in in_maps]
        concat_in = [np.concatenate([pc[i] for pc in per_core], axis=0)
                     for i in range(n_params)]
        concat_zeros = [np.zeros((n_cores * s[0], *s[1:]), d)
                        for s, d in out_shapes]
        out_arrs = sharded(*concat_in, *concat_zeros)
        return [{name: np.asarray(out_arrs[i]).reshape(n_cores, *out_shapes[i][0])[c]
                 for i, name in enumerate(out_names)}
                for c in range(n_cores)]

    run.sharded = sharded
    run.mesh = mesh
    run.in_names = in_names
    run.out_shapes = out_shapes
    run.n_cores = n_cores
    return run


def _get_runner(stop_after="full", reps=1):
    key = ("runner", stop_after, reps)
    if key not in _CACHE:
        _CACHE[key] = _make_runner(_build_nc(stop_after, reps))
    return _CACHE[key]


def _pad_image(logits, rel, anchors_pad):
    NPAD = NT * P
    L = np.zeros((NPAD, NC), np.float32); L[:8732] = logits
    R = np.zeros((NPAD, 4), np.float32); R[:8732] = rel
    return {"lg": L, "rl": R, "an": anchors_pad}


def _in_maps(bbox_regression, cls_logits, anchors):
    NPAD = NT * P
    A = np.tile(np.array([0, 0, 1, 1], np.float32), (NPAD, 1))
    A[:8732] = anchors
    B = cls_logits.shape[0]
    return [_pad_image(cls_logits[b], bbox_regression[b], A) for b in range(B)]


def _run(bbox_regression, cls_logits, anchors, stop_after="full", reps=1):
    res = _get_runner(stop_after, reps)(_in_maps(bbox_regression, cls_logits, anchors))
    out = np.stack([r["out"] for r in res]).astype(np.float32)
    return out, res


def kernel(bbox_regression, cls_logits, anchors):
    out, _ = _run(np.asarray(bbox_regression), np.asarray(cls_logits),
                  np.asarray(anchors))
    return out
